# revision 12
# baseline (speedup 1.0000x reference)
"""Trainium2 Bass kernel for nn_NVGPTDecoderLayer (single-token decode layer).

Tensor-parallel over 8 NeuronCores, vLLM style:
  - qkv column-parallel (4 heads/core), dense row-parallel
  - fc1 column-parallel, fc2 row-parallel (SwiGLU pairs interleaved host-side)
  - KV cache sharded along head dim; layernorm replicated
  - AllReduce after dense and after fc2

Attention tile counts are baked from `positions` at trace time (the program is
rebuilt per call, so this stays correct for whatever inputs are given).
"""
import os
import numpy as np

B, S, HID = 8, 4096, 4096
NH, HD, ROT = 32, 128, 64
FFN = 16384
EPS = 1e-5
SCALE = HD ** -0.5
NC_ = 8
HPC = NH // NC_          # 4 heads per core
QPC = HPC * HD           # 512
F1PC = 2 * FFN // NC_    # 4096 fc1 cols per core
F2PC = FFN // NC_        # 2048 fc2 rows per core


def _host_prep(positions):
    pos = np.asarray(positions).astype(np.int64)
    inv_freq = 1.0 / (10000.0 ** (np.arange(0, ROT, 2, dtype=np.float32) / ROT))
    ang = pos[:, None].astype(np.float32) * inv_freq[None, :]      # [8, 32]
    cos = np.cos(ang).astype(np.float32)
    sin = np.sin(ang).astype(np.float32)
    cosx_h = np.concatenate([cos, cos, np.ones((B, HD - ROT), np.float32)], 1)
    sinx_h = np.concatenate([-sin, sin, np.zeros((B, HD - ROT), np.float32)], 1)
    cosx = np.ascontiguousarray(np.tile(cosx_h, (1, HPC)))         # [8, 512]
    sinx = np.ascontiguousarray(np.tile(sinx_h, (1, HPC)))
    T = [int(np.ceil(p / 128)) if p > 0 else 0 for p in pos]
    maskT = np.zeros((128, B), np.float32)
    for b, p in enumerate(pos):
        t = T[b] - 1
        if t >= 0:
            jj = t * 128 + np.arange(128)
            maskT[:, b] = np.where(jj < p, 0.0, -1e30).astype(np.float32)
    return pos, cosx, sinx, T, maskT


def _shard_inputs(inputs):
    pos, cosx, sinx, T, maskT = _host_prep(inputs["positions"])
    h_np = np.ascontiguousarray(inputs["hidden_states"], dtype=np.float32)
    wq = inputs["w_qkv"]; wd = inputs["w_dense"]
    wf1 = inputs["w_fc1"]; wf2 = inputs["w_fc2"]
    lnT = {}
    for nm, w_, b_ in (("1", inputs["ln1_w"], inputs["ln1_b"]),
                       ("2", inputs["ln2_w"], inputs["ln2_b"])):
        lnT["w" + nm] = np.ascontiguousarray(
            (np.asarray(w_, dtype=np.float32) + 1.0).reshape(32, 128).T)
        lnT["b" + nm] = np.ascontiguousarray(
            np.asarray(b_, dtype=np.float32).reshape(32, 128).T)
    ident = np.eye(128, dtype=np.float32)
    in_maps = []
    for c in range(NC_):
        wq_c = np.ascontiguousarray(np.concatenate([
            wq[:, c * QPC:(c + 1) * QPC],
            wq[:, HID + c * QPC: HID + (c + 1) * QPC],
            wq[:, 2 * HID + c * QPC: 2 * HID + (c + 1) * QPC]], 1),
            dtype=np.float32)
        blocks = []
        for j in range(F1PC // 1024):
            o = c * (F1PC // 2) + j * 512
            blocks.append(wf1[:, o:o + 512])
            blocks.append(wf1[:, FFN + o: FFN + o + 512])
        wf1_c = np.ascontiguousarray(np.concatenate(blocks, 1), dtype=np.float32)
        m = {
            "hidden": h_np,
            "wqkv": wq_c,
            "wd": np.ascontiguousarray(wd[c * QPC:(c + 1) * QPC, :], dtype=np.float32),
            "wf1": wf1_c,
            "wf2": np.ascontiguousarray(wf2[c * F2PC:(c + 1) * F2PC, :], dtype=np.float32),
            "kc": np.ascontiguousarray(inputs["k_cache"][:, :, c * HPC:(c + 1) * HPC, :], dtype=np.float32),
            "vc": np.ascontiguousarray(inputs["v_cache"][:, :, c * HPC:(c + 1) * HPC, :], dtype=np.float32),
            "ln1wT": lnT["w1"], "ln1bT": lnT["b1"],
            "ln2wT": lnT["w2"], "ln2bT": lnT["b2"],
            "cosxq": (cosx * SCALE).astype(np.float32),
            "sinxq": (sinx * SCALE).astype(np.float32),
            "cosxk": cosx, "sinxk": sinx,
            "maskT": maskT, "ident": ident,
        }
        in_maps.append(m)
    return in_maps, T


def _build_program(T):
    import concourse.bass as bass  # noqa: F401
    import concourse.bacc as bacc
    import concourse.mybir as mybir
    import concourse.tile as tile

    F32 = mybir.dt.float32
    GEMM_DT = {"fp32": mybir.dt.float32, "fp32r": mybir.dt.float32r}[
        os.environ.get("KERNEL_GEMM_DT", "fp32r")]

    MDT = GEMM_DT

    def g(ap):
        return ap.bitcast(GEMM_DT)

    nc = bacc.Bacc("TRN2", target_bir_lowering=False, debug=False,
                   num_devices=NC_)

    def din(name, shape):
        return nc.dram_tensor(name, list(shape), F32, kind="ExternalInput").ap()

    hidden_d = din("hidden", (B, HID))
    wqkv_d = din("wqkv", (HID, 3 * QPC))
    wd_d = din("wd", (QPC, HID))
    wf1_d = din("wf1", (HID, F1PC))
    wf2_d = din("wf2", (F2PC, HID))
    kc_d = din("kc", (B, S, HPC, HD))
    vc_d = din("vc", (B, S, HPC, HD))
    ln1wT_d = din("ln1wT", (128, 32)); ln1bT_d = din("ln1bT", (128, 32))
    ln2wT_d = din("ln2wT", (128, 32)); ln2bT_d = din("ln2bT", (128, 32))
    cosxq_d = din("cosxq", (B, QPC)); sinxq_d = din("sinxq", (B, QPC))
    cosxk_d = din("cosxk", (B, QPC)); sinxk_d = din("sinxk", (B, QPC))
    maskT_d = din("maskT", (128, B))
    ident_d = din("ident", (128, 128))
    out_d = nc.dram_tensor("out", [B, HID], F32, kind="ExternalOutput").ap()

    AX = mybir.AxisListType.X
    OP = mybir.AluOpType
    AF = mybir.ActivationFunctionType

    with tile.TileContext(nc) as tc:
        with (tc.tile_pool(name="const", bufs=1) as cpool,
              tc.tile_pool(name="sbuf", bufs=1) as spool,
              tc.tile_pool(name="dram", bufs=1, space="DRAM") as dram):

            def cload(name, shape, src):
                t = cpool.tile(list(shape), F32, tag=name)
                nc.sync.dma_start(t[:], src[:])
                return t

            ident = cload("ident", (128, 128), ident_d)
            ln1wT = cload("ln1wT", (128, 32), ln1wT_d)
            ln1bT = cload("ln1bT", (128, 32), ln1bT_d)
            ln2wT = cload("ln2wT", (128, 32), ln2wT_d)
            ln2bT = cload("ln2bT", (128, 32), ln2bT_d)
            cosxq = cload("cosxq", (B, QPC), cosxq_d)
            sinxq = cload("sinxq", (B, QPC), sinxq_d)
            cosxk = cload("cosxk", (B, QPC), cosxk_d)
            sinxk = cload("sinxk", (B, QPC), sinxk_d)
            maskT = cload("maskT", (128, B), maskT_d)
            ones_col = cpool.tile([128, 1], F32, tag="ones_col")
            nc.vector.memset(ones_col[:], 1.0)
            ones_row = cpool.tile([1, 128], F32, tag="ones_row")
            nc.vector.memset(ones_row[:], 1.0)
            zbias = cpool.tile([128, 1], F32, tag="zbias")
            nc.vector.memset(zbias[:], 0.0)
            ebias = cpool.tile([128, 1], F32, tag="ebias")
            nc.vector.memset(ebias[:], EPS)

            hidden = spool.tile([B, HID], F32, tag="hidden")
            nc.sync.dma_start(hidden[:], hidden_d[:])

            def ln_stats(x_sb):
                st = spool.tile([B, HID], F32, tag="lnst")
                ssum = spool.tile([B, 1], F32, tag="ln_sum")
                sqs = spool.tile([B, 1], F32, tag="ln_sqs")
                nc.vector.reduce_sum(out=ssum[:], in_=x_sb[:], axis=AX)
                nc.scalar.activation(st[:], x_sb[:], AF.Square,
                                     bias=zbias[0:B, :], accum_out=sqs[:])
                nmu = spool.tile([B, 1], F32, tag="ln_nmu")
                nc.vector.tensor_scalar_mul(nmu[:], ssum[:], -1.0 / HID)
                ex2 = spool.tile([B, 1], F32, tag="ln_ex2")
                nc.vector.tensor_scalar_mul(ex2[:], sqs[:], 1.0 / HID)
                mu2 = spool.tile([B, 1], F32, tag="ln_mu2")
                nc.vector.tensor_tensor(out=mu2[:], in0=nmu[:], in1=nmu[:],
                                        op=OP.mult)
                var = spool.tile([B, 1], F32, tag="ln_var")
                nc.vector.tensor_tensor(out=var[:], in0=ex2[:], in1=mu2[:],
                                        op=OP.subtract)
                sd = spool.tile([B, 1], F32, tag="ln_sd")
                nc.scalar.activation(sd[:], var[:], AF.Sqrt, bias=ebias[0:B, :])
                rstd = spool.tile([B, 1], F32, tag="ln_rstd")
                nc.vector.reciprocal(rstd[:], sd[:])
                xc = spool.tile([B, HID], F32, tag="ln_xc")
                nc.vector.tensor_scalar(out=xc[:], in0=x_sb[:],
                                        scalar1=nmu[:], scalar2=rstd[:],
                                        op0=OP.add, op1=OP.mult)
                return xc

            def ln_transpose(xc, wT, bT, tpool, out_tag):
                xT = spool.tile([128, 32 * B], MDT, tag=out_tag)
                for half in range(2):
                    tp = tpool.tile([128, 16 * B], F32, tag="tp")
                    for i2 in range(16):
                        i = half * 16 + i2
                        nc.tensor.matmul(
                            tp[:, i2 * B:(i2 + 1) * B],
                            xc[:, i * 128:(i + 1) * 128],
                            ident[0:B, 0:B], is_transpose=True,
                            start=True, stop=True)
                    for i2 in range(16):
                        i = half * 16 + i2
                        nc.vector.tensor_scalar(
                            out=xT[:, i * B:(i + 1) * B],
                            in0=tp[:, i2 * B:(i2 + 1) * B],
                            scalar1=wT[:, i:i + 1], scalar2=bT[:, i:i + 1],
                            op0=OP.mult, op1=OP.add)
                return xT

            def rope(ps, cosx_t, sinx_t, tag):
                r = spool.tile([B, QPC], F32, tag=tag)
                nc.vector.tensor_tensor(out=r[:], in0=ps[:], in1=cosx_t[:],
                                        op=OP.mult)
                t2 = spool.tile([B, HPC * ROT], F32, tag=tag + "_t2")
                ps_r = ps[:].rearrange("b (h r) -> b h r", h=HPC)
                sx_r = sinx_t[:].rearrange("b (h r) -> b h r", h=HPC)
                t2_r = t2[:].rearrange("b (h r) -> b h r", h=HPC)
                nc.vector.tensor_tensor(out=t2_r[:, :, 0:32],
                                        in0=ps_r[:, :, 32:64],
                                        in1=sx_r[:, :, 0:32], op=OP.mult)
                nc.vector.tensor_tensor(out=t2_r[:, :, 32:64],
                                        in0=ps_r[:, :, 0:32],
                                        in1=sx_r[:, :, 32:64], op=OP.mult)
                r_r = r[:].rearrange("b (h r) -> b h r", h=HPC)
                nc.vector.tensor_tensor(out=r_r[:, :, 0:ROT],
                                        in0=r_r[:, :, 0:ROT], in1=t2_r[:],
                                        op=OP.add)
                return r

            # ============ phase 1: LN1 + QKV + RoPE + qT ============
            with (tc.tile_pool(name="ps_tp1", bufs=2, space="PSUM") as tp1,
                  tc.tile_pool(name="ps_qkv", bufs=1, space="PSUM") as ps_qkv,
                  tc.tile_pool(name="wq", bufs=2) as wq_pool):
                xc1 = ln_stats(hidden)
                x1T = ln_transpose(xc1, ln1wT, ln1bT, tp1, "x1T")

                qkv_ps = [ps_qkv.tile([B, 512], F32, tag=f"qkv{n}",
                                      name=f"qkv_ps{n}") for n in range(3)]
                for ki in range(32):
                    wq_sb = wq_pool.tile([128, 3 * QPC], MDT, tag="wq")
                    nc.sync.dma_start(wq_sb[:],
                                      g(wqkv_d[ki * 128:(ki + 1) * 128, :]))
                    for n in range(3):
                        nc.tensor.matmul(
                            qkv_ps[n][:], x1T[:, ki * B:(ki + 1) * B],
                            wq_sb[:, n * 512:(n + 1) * 512],
                            start=(ki == 0), stop=(ki == 31))

                qr = rope(qkv_ps[0], cosxq, sinxq, "qr")
                kr = rope(qkv_ps[1], cosxk, sinxk, "kr")
                v_sb = spool.tile([B, QPC], F32, tag="v_sb")
                nc.vector.tensor_copy(v_sb[:], qkv_ps[2][:])

                qT_ps = tp1.tile([128, 32], F32, tag="tp")
                for h in range(HPC):
                    nc.tensor.matmul(
                        qT_ps[:].rearrange("p (b h) -> p b h", h=HPC)[:, :, h],
                        qr[:, h * HD:(h + 1) * HD], ident[0:B, 0:B],
                        is_transpose=True, start=True, stop=True)
                qT = spool.tile([128, 32], F32, tag="qT")
                nc.vector.tensor_copy(qT[:], qT_ps[:])

                tmp_qk = spool.tile([B, QPC], F32, tag="tmp_qk")
                nc.vector.tensor_tensor(out=tmp_qk[:], in0=qr[:], in1=kr[:],
                                        op=OP.mult)
                s_new = spool.tile([B, HPC], F32, tag="s_new")
                nc.vector.reduce_sum(
                    out=s_new[:],
                    in_=tmp_qk[:].rearrange("b (h d) -> b h d", h=HPC),
                    axis=AX)
                e_new = spool.tile([B, HPC], F32, tag="e_new")
                nc.scalar.activation(e_new[:], s_new[:], AF.Exp, bias=zbias[0:B, :])

                vw = spool.tile([B, QPC], F32, tag="vw")
                for h in range(HPC):
                    nc.vector.tensor_scalar_mul(
                        vw[:, h * HD:(h + 1) * HD],
                        v_sb[:, h * HD:(h + 1) * HD], e_new[:, h:h + 1])

            # ============ phase 2: attention ============
            with (tc.tile_pool(name="ps_attn", bufs=1, space="PSUM") as ps_attn,
                  tc.tile_pool(name="exp", bufs=4) as exp_pool):
                attn_ps = ps_attn.tile([128, 32], F32, tag="attn")
                nc.vector.memset(attn_ps[:], 0.0)
                denom_lo = ps_attn.tile([1, 512], F32, tag="denom_lo")
                nc.vector.memset(denom_lo[:], 0.0)
                denom_hi = ps_attn.tile([1, 512], F32, tag="denom_hi")
                nc.vector.memset(denom_hi[:], 0.0)

                attn_loop = (
                  tc.tile_pool(name="ps_kt", bufs=2, space="PSUM"),
                  tc.tile_pool(name="ps_sc", bufs=2, space="PSUM"),
                  tc.tile_pool(name="kv", bufs=3),
                  tc.tile_pool(name="kv_v", bufs=4),
                  tc.tile_pool(name="ktsb", bufs=3))
                ps_kt, ps_sc, kv_pool, kvv_pool, kt_pool = (
                    p.__enter__() for p in attn_loop)
                ps_kt, ps_sc, kv_pool, kvv_pool, kt_pool = [
                    p for p in (ps_kt, ps_sc, kv_pool, kvv_pool, kt_pool)]

                for b in range(B):
                    Tb = T[b]
                    if Tb == 0:
                        continue
                    for h in range(HPC):
                        m = b * HPC + h
                        sc_ps = ps_sc.tile([128, 32], F32, tag="sc")
                        n_ch = (Tb + 3) // 4
                        for ci in range(n_ch):
                            nt = min(4, Tb - ci * 4)
                            k_sb = kv_pool.tile([128, 512], F32, tag="k_sb")
                            nc.sync.dma_start(
                                k_sb[:, 0:nt * 128]
                                .rearrange("p (i d) -> p i d", i=nt),
                                kc_d[b, ci * 512:ci * 512 + nt * 128, h, :]
                                .rearrange("(i p) d -> p i d", p=128))
                            for j in range(nt):
                                t = ci * 4 + j
                                kt_ps = ps_kt.tile([128, 128], F32, tag="kt")
                                nc.tensor.matmul(
                                    kt_ps[:], k_sb[:, j * 128:(j + 1) * 128],
                                    ident[:, :], is_transpose=True,
                                    start=True, stop=True)
                                kt_sb = kt_pool.tile([128, 128], F32,
                                                     tag="kt_sb")
                                nc.vector.tensor_copy(kt_sb[:], kt_ps[:])
                                nc.tensor.matmul(
                                    sc_ps[:, t:t + 1], kt_sb[:],
                                    qT[:, m:m + 1], start=True, stop=True)
                        exp_sb = exp_pool.tile([128, 32], F32, tag="exp")
                        if Tb > 1:
                            nc.scalar.activation(exp_sb[:, 0:Tb - 1],
                                                 sc_ps[:, 0:Tb - 1], AF.Exp,
                                                 bias=zbias[:, :])
                        nc.scalar.activation(exp_sb[:, Tb - 1:Tb],
                                             sc_ps[:, Tb - 1:Tb], AF.Exp,
                                             bias=maskT[:, b:b + 1])
                        dtile = denom_lo if m < 16 else denom_hi
                        mm = m % 16
                        nc.tensor.matmul(
                            dtile[0:1, mm * 32:mm * 32 + Tb],
                            ones_col[:], exp_sb[:, 0:Tb],
                            start=True, stop=True)
                        for ci in range(n_ch):
                            nt = min(4, Tb - ci * 4)
                            v_sb_t = kvv_pool.tile([128, 512], F32,
                                                   tag="v_sbt")
                            nc.sync.dma_start(
                                v_sb_t[:, 0:nt * 128]
                                .rearrange("p (i d) -> p i d", i=nt),
                                vc_d[b, ci * 512:ci * 512 + nt * 128, h, :]
                                .rearrange("(i p) d -> p i d", p=128))
                            for j in range(nt):
                                t = ci * 4 + j
                                nc.tensor.matmul(
                                    attn_ps[:, m:m + 1],
                                    v_sb_t[:, j * 128:(j + 1) * 128],
                                    exp_sb[:, t:t + 1],
                                    start=False, stop=(t == Tb - 1))

                for p in reversed(attn_loop):
                    p.__exit__(None, None, None)

                # new-token contribution + denominators + normalize
                with tc.tile_pool(name="ps_attn2", bufs=1,
                                  space="PSUM") as ps_attn2:
                    attn_new_ps = ps_attn2.tile([128, 32], F32, tag="attn_new")
                    for h in range(HPC):
                        nc.tensor.matmul(
                            attn_new_ps[:].rearrange(
                                "p (b h) -> p b h", h=HPC)[:, :, h],
                            vw[:, h * HD:(h + 1) * HD], ident[0:B, 0:B],
                            is_transpose=True, start=True, stop=True)
                    denom_flat = spool.tile([1, 32], F32, tag="denom_flat")
                    nc.vector.reduce_sum(
                        out=denom_flat[0:1, 0:16],
                        in_=denom_lo[:].rearrange("o (m t) -> o m t", t=32),
                        axis=AX)
                    nc.vector.reduce_sum(
                        out=denom_flat[0:1, 16:32],
                        in_=denom_hi[:].rearrange("o (m t) -> o m t", t=32),
                        axis=AX)
                    enew_dram = dram.tile([B, HPC], F32, tag="enew_d")
                    nc.sync.dma_start(enew_dram[:], e_new[:])
                    enew_flat = spool.tile([1, 32], F32, tag="enew_flat")
                    nc.sync.dma_start(
                        enew_flat[:],
                        enew_dram[:].rearrange("b h -> () (b h)"))
                    nc.vector.tensor_tensor(out=denom_flat[:],
                                            in0=denom_flat[:],
                                            in1=enew_flat[:], op=OP.add)
                    recip_flat = spool.tile([1, 32], F32, tag="recip_flat")
                    nc.vector.reciprocal(recip_flat[:], denom_flat[:])
                    bcast_ps = ps_attn2.tile([128, 32], F32, tag="bcast")
                    nc.tensor.matmul(bcast_ps[:], ones_row[:], recip_flat[:],
                                     start=True, stop=True)
                    attn_tot = spool.tile([128, 32], F32, tag="attn_tot")
                    nc.vector.tensor_copy(attn_tot[:], attn_ps[:])
                    nc.vector.tensor_tensor(out=attn_tot[:], in0=attn_tot[:],
                                            in1=attn_new_ps[:], op=OP.add)
                    attn_n = spool.tile([128, 32], MDT, tag="attn_n")
                    nc.vector.tensor_tensor(out=attn_n[:], in0=attn_tot[:],
                                            in1=bcast_ps[:], op=OP.mult)

            # ============ phase 3: dense + AR1 ============
            ar1_in = dram.tile([B, HID], F32, tag="ar1_in")
            ar1_out = dram.tile([B, HID], F32, tag="ar1_out")
            attn_nr = attn_n[:].rearrange("p (b h) -> p b h", h=HPC)
            with (tc.tile_pool(name="ps_o8a", bufs=8, space="PSUM") as o8a,
                  tc.tile_pool(name="wdp", bufs=2) as wdp):
                dps = [o8a.tile([B, 512], F32, tag="o8", name=f"dps{n}")
                       for n in range(8)]
                for h in range(HPC):
                    wd_sb = wdp.tile([128, HID], MDT, tag="wd")
                    nc.sync.dma_start(wd_sb[:],
                                      g(wd_d[h * 128:(h + 1) * 128, :]))
                    for n in range(8):
                        nc.tensor.matmul(
                            dps[n][:], attn_nr[:, :, h],
                            wd_sb[:, n * 512:(n + 1) * 512],
                            start=(h == 0), stop=(h == HPC - 1))
                gout = spool.tile([B, HID], F32, tag="gemmout")
                for n in range(8):
                    nc.vector.tensor_copy(gout[:, n * 512:(n + 1) * 512],
                                          dps[n][:])
                nc.sync.dma_start(ar1_in[:], gout[:])
            nc.gpsimd.collective_compute(
                "AllReduce", mybir.AluOpType.add,
                replica_groups=[list(range(NC_))],
                ins=[ar1_in.opt()], outs=[ar1_out.opt()])

            x2 = spool.tile([B, HID], F32, tag="x2")
            ar_sb = spool.tile([B, HID], F32, tag="ar_sb")
            nc.sync.dma_start(ar_sb[:], ar1_out[:])
            nc.vector.tensor_tensor(out=x2[:], in0=hidden[:], in1=ar_sb[:],
                                    op=OP.add)

            # ============ phase 4: LN2 + FFN + AR2 ============
            hg = spool.tile([B, F1PC // 2], F32, tag="hg")
            with (tc.tile_pool(name="ps_tp2", bufs=2, space="PSUM") as tp2,
                  tc.tile_pool(name="ps_o8b", bufs=4, space="PSUM") as o8b,
                  tc.tile_pool(name="wf1p", bufs=3) as wf1p):
                xc2 = ln_stats(x2)
                x2T = ln_transpose(xc2, ln2wT, ln2bT, tp2, "x2T")
                for pair in range(4):
                    hp = [o8b.tile([B, 512], F32, tag="o8",
                                   name=f"hp{pair}_{n}") for n in range(2)]
                    for ki in range(32):
                        w_sb = wf1p.tile([128, 1024], MDT, tag="wf1c")
                        nc.sync.dma_start(
                            w_sb[:],
                            g(wf1_d[ki * 128:(ki + 1) * 128,
                                    pair * 1024:(pair + 1) * 1024]))
                        for n in range(2):
                            nc.tensor.matmul(
                                hp[n][:], x2T[:, ki * B:(ki + 1) * B],
                                w_sb[:, n * 512:(n + 1) * 512],
                                start=(ki == 0), stop=(ki == 31))
                    sil = spool.tile([B, 512], F32, tag="sil")
                    nc.scalar.activation(sil[:], hp[0][:], AF.Silu,
                                         bias=zbias[0:B, :])
                    nc.vector.tensor_tensor(
                        out=hg[:, pair * 512:(pair + 1) * 512],
                        in0=sil[:], in1=hp[1][:], op=OP.mult)

                hT = spool.tile([128, 16 * B], MDT, tag="hT")
                tpt = tp2.tile([128, 16 * B], F32, tag="tp")
                for i in range(16):
                    nc.tensor.matmul(
                        tpt[:, i * B:(i + 1) * B],
                        hg[:, i * 128:(i + 1) * 128], ident[0:B, 0:B],
                        is_transpose=True, start=True, stop=True)
                nc.vector.tensor_copy(hT[:], tpt[:])

            ar2_in = dram.tile([B, HID], F32, tag="ar2_in")
            ar2_out = dram.tile([B, HID], F32, tag="ar2_out")
            with (tc.tile_pool(name="ps_o8c", bufs=8, space="PSUM") as o8c,
                  tc.tile_pool(name="wf2p", bufs=2) as wf2p):
                fps = [o8c.tile([B, 512], F32, tag="o8", name=f"fps{n}")
                       for n in range(8)]
                for ki in range(16):
                    w_sb = wf2p.tile([128, HID], MDT, tag="wf2c")
                    nc.sync.dma_start(w_sb[:],
                                      g(wf2_d[ki * 128:(ki + 1) * 128, :]))
                    for n in range(8):
                        nc.tensor.matmul(
                            fps[n][:], hT[:, ki * B:(ki + 1) * B],
                            w_sb[:, n * 512:(n + 1) * 512],
                            start=(ki == 0), stop=(ki == 15))
                gout2 = spool.tile([B, HID], F32, tag="gemmout")
                for n in range(8):
                    nc.vector.tensor_copy(gout2[:, n * 512:(n + 1) * 512],
                                          fps[n][:])
                nc.sync.dma_start(ar2_in[:], gout2[:])
            nc.gpsimd.collective_compute(
                "AllReduce", mybir.AluOpType.add,
                replica_groups=[list(range(NC_))],
                ins=[ar2_in.opt()], outs=[ar2_out.opt()])

            ar2_sb = spool.tile([B, HID], F32, tag="ar2_sb")
            nc.sync.dma_start(ar2_sb[:], ar2_out[:])
            out_sb = spool.tile([B, HID], F32, tag="out_sb")
            nc.vector.tensor_tensor(out=out_sb[:], in0=x2[:], in1=ar2_sb[:],
                                    op=OP.add)
            nc.sync.dma_start(out_d[:], out_sb[:])

    nc.compile()
    return nc


def kernel(**inputs):
    from concourse.bass_utils import run_bass_kernel_spmd
    in_maps, T = _shard_inputs(inputs)
    nc = _build_program(T)
    trace = os.environ.get("KERNEL_TRACE", "0") == "1"
    tdir = os.environ.get("KERNEL_TRACE_DIR") or None
    res = run_bass_kernel_spmd(nc, in_maps, list(range(NC_)), trace=trace,
                               tmpdir=tdir)
    if trace and res.exec_time_ns is not None:
        print(f"HW exec time: {res.exec_time_ns} ns")
        print(f"mean exec time: {res.mean_exec_time_ns} ns "
              f"(max core {res.max_exec_time_core_id})")
        kernel.last_exec_time_ns = res.exec_time_ns
    return res.results[0]["out"]


if __name__ == "__main__":
    data = np.load("/tmp/ref_inputs.npz")
    inputs = {k: data[k] for k in data.files}
    ref = np.load("/tmp/ref_out.npy")
    out = kernel(**inputs)
    err = np.abs(out - ref).max()
    rel = err / np.abs(ref).max()
    print(f"absmax err: {err:.3e}  rel: {rel:.3e}")


# revision 13
# speedup vs baseline: 1.0087x; 1.0087x over previous
"""Trainium2 Bass kernel for nn_NVGPTDecoderLayer (single-token decode layer).

Tensor-parallel over 8 NeuronCores, vLLM style:
  - qkv column-parallel (4 heads/core), dense row-parallel
  - fc1 column-parallel, fc2 row-parallel (SwiGLU pairs interleaved host-side)
  - KV cache sharded along head dim; layernorm replicated
  - AllReduce after dense and after fc2

Attention tile counts are baked from `positions` at trace time (the program is
rebuilt per call, so this stays correct for whatever inputs are given).
"""
import os
import numpy as np

B, S, HID = 8, 4096, 4096
NH, HD, ROT = 32, 128, 64
FFN = 16384
EPS = 1e-5
SCALE = HD ** -0.5
NC_ = 8
HPC = NH // NC_          # 4 heads per core
QPC = HPC * HD           # 512
F1PC = 2 * FFN // NC_    # 4096 fc1 cols per core
F2PC = FFN // NC_        # 2048 fc2 rows per core


def _host_prep(positions):
    pos = np.asarray(positions).astype(np.int64)
    inv_freq = 1.0 / (10000.0 ** (np.arange(0, ROT, 2, dtype=np.float32) / ROT))
    ang = pos[:, None].astype(np.float32) * inv_freq[None, :]      # [8, 32]
    cos = np.cos(ang).astype(np.float32)
    sin = np.sin(ang).astype(np.float32)
    cosx_h = np.concatenate([cos, cos, np.ones((B, HD - ROT), np.float32)], 1)
    sinx_h = np.concatenate([-sin, sin, np.zeros((B, HD - ROT), np.float32)], 1)
    cosx = np.ascontiguousarray(np.tile(cosx_h, (1, HPC)))         # [8, 512]
    sinx = np.ascontiguousarray(np.tile(sinx_h, (1, HPC)))
    T = [int(np.ceil(p / 128)) if p > 0 else 0 for p in pos]
    maskT = np.zeros((128, B), np.float32)
    for b, p in enumerate(pos):
        t = T[b] - 1
        if t >= 0:
            jj = t * 128 + np.arange(128)
            maskT[:, b] = np.where(jj < p, 0.0, -1e30).astype(np.float32)
    return pos, cosx, sinx, T, maskT


def _shard_inputs(inputs):
    pos, cosx, sinx, T, maskT = _host_prep(inputs["positions"])
    h_np = np.ascontiguousarray(inputs["hidden_states"], dtype=np.float32)
    wq = inputs["w_qkv"]; wd = inputs["w_dense"]
    wf1 = inputs["w_fc1"]; wf2 = inputs["w_fc2"]
    lnT = {}
    for nm, w_, b_ in (("1", inputs["ln1_w"], inputs["ln1_b"]),
                       ("2", inputs["ln2_w"], inputs["ln2_b"])):
        lnT["w" + nm] = np.ascontiguousarray(
            (np.asarray(w_, dtype=np.float32) + 1.0).reshape(32, 128).T)
        lnT["b" + nm] = np.ascontiguousarray(
            np.asarray(b_, dtype=np.float32).reshape(32, 128).T)
    ident = np.eye(128, dtype=np.float32)
    in_maps = []
    for c in range(NC_):
        wq_c = np.ascontiguousarray(np.concatenate([
            wq[:, c * QPC:(c + 1) * QPC],
            wq[:, HID + c * QPC: HID + (c + 1) * QPC],
            wq[:, 2 * HID + c * QPC: 2 * HID + (c + 1) * QPC]], 1),
            dtype=np.float32)
        blocks = []
        for j in range(F1PC // 1024):
            o = c * (F1PC // 2) + j * 512
            blocks.append(wf1[:, o:o + 512])
            blocks.append(wf1[:, FFN + o: FFN + o + 512])
        wf1_c = np.ascontiguousarray(np.concatenate(blocks, 1), dtype=np.float32)
        m = {
            "hidden": h_np,
            "wqkv": wq_c,
            "wd": np.ascontiguousarray(wd[c * QPC:(c + 1) * QPC, :], dtype=np.float32),
            "wf1": wf1_c,
            "wf2": np.ascontiguousarray(wf2[c * F2PC:(c + 1) * F2PC, :], dtype=np.float32),
            "kc": np.ascontiguousarray(inputs["k_cache"][:, :, c * HPC:(c + 1) * HPC, :], dtype=np.float32),
            "vc": np.ascontiguousarray(inputs["v_cache"][:, :, c * HPC:(c + 1) * HPC, :], dtype=np.float32),
            "ln1wT": lnT["w1"], "ln1bT": lnT["b1"],
            "ln2wT": lnT["w2"], "ln2bT": lnT["b2"],
            "cosxq": (cosx * SCALE).astype(np.float32),
            "sinxq": (sinx * SCALE).astype(np.float32),
            "cosxk": cosx, "sinxk": sinx,
            "maskT": maskT, "ident": ident,
        }
        in_maps.append(m)
    return in_maps, T


def _build_program(T):
    import concourse.bass as bass  # noqa: F401
    import concourse.bacc as bacc
    import concourse.mybir as mybir
    import concourse.tile as tile

    F32 = mybir.dt.float32
    GEMM_DT = {"fp32": mybir.dt.float32, "fp32r": mybir.dt.float32r}[
        os.environ.get("KERNEL_GEMM_DT", "fp32r")]

    MDT = GEMM_DT

    def g(ap):
        return ap.bitcast(GEMM_DT)

    nc = bacc.Bacc("TRN2", target_bir_lowering=False, debug=False,
                   num_devices=NC_)

    def din(name, shape):
        return nc.dram_tensor(name, list(shape), F32, kind="ExternalInput").ap()

    hidden_d = din("hidden", (B, HID))
    wqkv_d = din("wqkv", (HID, 3 * QPC))
    wd_d = din("wd", (QPC, HID))
    wf1_d = din("wf1", (HID, F1PC))
    wf2_d = din("wf2", (F2PC, HID))
    kc_d = din("kc", (B, S, HPC, HD))
    vc_d = din("vc", (B, S, HPC, HD))
    ln1wT_d = din("ln1wT", (128, 32)); ln1bT_d = din("ln1bT", (128, 32))
    ln2wT_d = din("ln2wT", (128, 32)); ln2bT_d = din("ln2bT", (128, 32))
    cosxq_d = din("cosxq", (B, QPC)); sinxq_d = din("sinxq", (B, QPC))
    cosxk_d = din("cosxk", (B, QPC)); sinxk_d = din("sinxk", (B, QPC))
    maskT_d = din("maskT", (128, B))
    ident_d = din("ident", (128, 128))
    out_d = nc.dram_tensor("out", [B, HID], F32, kind="ExternalOutput").ap()

    AX = mybir.AxisListType.X
    OP = mybir.AluOpType
    AF = mybir.ActivationFunctionType

    with tile.TileContext(nc) as tc:
        with (tc.tile_pool(name="const", bufs=1) as cpool,
              tc.tile_pool(name="sbuf", bufs=1) as spool,
              tc.tile_pool(name="dram", bufs=1, space="DRAM") as dram):

            def cload(name, shape, src):
                t = cpool.tile(list(shape), F32, tag=name)
                nc.sync.dma_start(t[:], src[:])
                return t

            ident = cload("ident", (128, 128), ident_d)
            ln1wT = cload("ln1wT", (128, 32), ln1wT_d)
            ln1bT = cload("ln1bT", (128, 32), ln1bT_d)
            ln2wT = cload("ln2wT", (128, 32), ln2wT_d)
            ln2bT = cload("ln2bT", (128, 32), ln2bT_d)
            cosxq = cload("cosxq", (B, QPC), cosxq_d)
            sinxq = cload("sinxq", (B, QPC), sinxq_d)
            cosxk = cload("cosxk", (B, QPC), cosxk_d)
            sinxk = cload("sinxk", (B, QPC), sinxk_d)
            maskT = cload("maskT", (128, B), maskT_d)
            ones_col = cpool.tile([128, 1], F32, tag="ones_col")
            nc.vector.memset(ones_col[:], 1.0)
            ones_row = cpool.tile([1, 128], F32, tag="ones_row")
            nc.vector.memset(ones_row[:], 1.0)
            zbias = cpool.tile([128, 1], F32, tag="zbias")
            nc.vector.memset(zbias[:], 0.0)
            ebias = cpool.tile([128, 1], F32, tag="ebias")
            nc.vector.memset(ebias[:], EPS)

            hidden = spool.tile([B, HID], F32, tag="hidden")
            nc.sync.dma_start(hidden[:], hidden_d[:])

            def ln_stats(x_sb):
                st = spool.tile([B, HID], F32, tag="lnst")
                ssum = spool.tile([B, 1], F32, tag="ln_sum")
                sqs = spool.tile([B, 1], F32, tag="ln_sqs")
                nc.vector.reduce_sum(out=ssum[:], in_=x_sb[:], axis=AX)
                nc.scalar.activation(st[:], x_sb[:], AF.Square,
                                     bias=zbias[0:B, :], accum_out=sqs[:])
                nmu = spool.tile([B, 1], F32, tag="ln_nmu")
                nc.vector.tensor_scalar_mul(nmu[:], ssum[:], -1.0 / HID)
                ex2 = spool.tile([B, 1], F32, tag="ln_ex2")
                nc.vector.tensor_scalar_mul(ex2[:], sqs[:], 1.0 / HID)
                mu2 = spool.tile([B, 1], F32, tag="ln_mu2")
                nc.vector.tensor_tensor(out=mu2[:], in0=nmu[:], in1=nmu[:],
                                        op=OP.mult)
                var = spool.tile([B, 1], F32, tag="ln_var")
                nc.vector.tensor_tensor(out=var[:], in0=ex2[:], in1=mu2[:],
                                        op=OP.subtract)
                sd = spool.tile([B, 1], F32, tag="ln_sd")
                nc.scalar.activation(sd[:], var[:], AF.Sqrt, bias=ebias[0:B, :])
                rstd = spool.tile([B, 1], F32, tag="ln_rstd")
                nc.vector.reciprocal(rstd[:], sd[:])
                xc = spool.tile([B, HID], F32, tag="ln_xc")
                nc.vector.tensor_scalar(out=xc[:], in0=x_sb[:],
                                        scalar1=nmu[:], scalar2=rstd[:],
                                        op0=OP.add, op1=OP.mult)
                return xc

            def ln_transpose(xc, wT, bT, tpool, out_tag):
                xT = spool.tile([128, 32 * B], MDT, tag=out_tag)
                for half in range(2):
                    tp = tpool.tile([128, 16 * B], F32, tag="tp")
                    for i2 in range(16):
                        i = half * 16 + i2
                        nc.tensor.matmul(
                            tp[:, i2 * B:(i2 + 1) * B],
                            xc[:, i * 128:(i + 1) * 128],
                            ident[0:B, 0:B], is_transpose=True,
                            start=True, stop=True)
                    for i2 in range(16):
                        i = half * 16 + i2
                        nc.vector.tensor_scalar(
                            out=xT[:, i * B:(i + 1) * B],
                            in0=tp[:, i2 * B:(i2 + 1) * B],
                            scalar1=wT[:, i:i + 1], scalar2=bT[:, i:i + 1],
                            op0=OP.mult, op1=OP.add)
                return xT

            def rope(ps, cosx_t, sinx_t, tag):
                r = spool.tile([B, QPC], F32, tag=tag)
                nc.vector.tensor_tensor(out=r[:], in0=ps[:], in1=cosx_t[:],
                                        op=OP.mult)
                t2 = spool.tile([B, HPC * ROT], F32, tag=tag + "_t2")
                ps_r = ps[:].rearrange("b (h r) -> b h r", h=HPC)
                sx_r = sinx_t[:].rearrange("b (h r) -> b h r", h=HPC)
                t2_r = t2[:].rearrange("b (h r) -> b h r", h=HPC)
                nc.vector.tensor_tensor(out=t2_r[:, :, 0:32],
                                        in0=ps_r[:, :, 32:64],
                                        in1=sx_r[:, :, 0:32], op=OP.mult)
                nc.vector.tensor_tensor(out=t2_r[:, :, 32:64],
                                        in0=ps_r[:, :, 0:32],
                                        in1=sx_r[:, :, 32:64], op=OP.mult)
                r_r = r[:].rearrange("b (h r) -> b h r", h=HPC)
                nc.vector.tensor_tensor(out=r_r[:, :, 0:ROT],
                                        in0=r_r[:, :, 0:ROT], in1=t2_r[:],
                                        op=OP.add)
                return r

            # ============ phase 1: LN1 + QKV + RoPE + qT ============
            with (tc.tile_pool(name="ps_tp1", bufs=2, space="PSUM") as tp1,
                  tc.tile_pool(name="ps_qkv", bufs=1, space="PSUM") as ps_qkv,
                  tc.tile_pool(name="wq", bufs=2) as wq_pool):
                xc1 = ln_stats(hidden)
                x1T = ln_transpose(xc1, ln1wT, ln1bT, tp1, "x1T")

                qkv_ps = [ps_qkv.tile([B, 512], F32, tag=f"qkv{n}",
                                      name=f"qkv_ps{n}") for n in range(3)]
                for ki in range(32):
                    wq_sb = wq_pool.tile([128, 3 * QPC], MDT, tag="wq")
                    nc.sync.dma_start(wq_sb[:],
                                      g(wqkv_d[ki * 128:(ki + 1) * 128, :]))
                    for n in range(3):
                        nc.tensor.matmul(
                            qkv_ps[n][:], x1T[:, ki * B:(ki + 1) * B],
                            wq_sb[:, n * 512:(n + 1) * 512],
                            start=(ki == 0), stop=(ki == 31))

                qr = rope(qkv_ps[0], cosxq, sinxq, "qr")
                kr = rope(qkv_ps[1], cosxk, sinxk, "kr")
                v_sb = spool.tile([B, QPC], F32, tag="v_sb")
                nc.vector.tensor_copy(v_sb[:], qkv_ps[2][:])

                qT_ps = tp1.tile([128, 32], F32, tag="tp")
                for h in range(HPC):
                    nc.tensor.matmul(
                        qT_ps[:].rearrange("p (b h) -> p b h", h=HPC)[:, :, h],
                        qr[:, h * HD:(h + 1) * HD], ident[0:B, 0:B],
                        is_transpose=True, start=True, stop=True)
                qT = spool.tile([128, 32], F32, tag="qT")
                nc.vector.tensor_copy(qT[:], qT_ps[:])

                tmp_qk = spool.tile([B, QPC], F32, tag="tmp_qk")
                nc.vector.tensor_tensor(out=tmp_qk[:], in0=qr[:], in1=kr[:],
                                        op=OP.mult)
                s_new = spool.tile([B, HPC], F32, tag="s_new")
                nc.vector.reduce_sum(
                    out=s_new[:],
                    in_=tmp_qk[:].rearrange("b (h d) -> b h d", h=HPC),
                    axis=AX)
                e_new = spool.tile([B, HPC], F32, tag="e_new")
                nc.scalar.activation(e_new[:], s_new[:], AF.Exp, bias=zbias[0:B, :])

                vw = spool.tile([B, QPC], F32, tag="vw")
                for h in range(HPC):
                    nc.vector.tensor_scalar_mul(
                        vw[:, h * HD:(h + 1) * HD],
                        v_sb[:, h * HD:(h + 1) * HD], e_new[:, h:h + 1])

            # ============ phase 2: attention ============
            with (tc.tile_pool(name="ps_attn", bufs=1, space="PSUM") as ps_attn,
                  tc.tile_pool(name="exp", bufs=4) as exp_pool):
                attn_ps = ps_attn.tile([128, 32], F32, tag="attn")
                nc.vector.memset(attn_ps[:], 0.0)

                attn_loop = (
                  tc.tile_pool(name="ps_kt", bufs=2, space="PSUM"),
                  tc.tile_pool(name="ps_sc", bufs=2, space="PSUM"),
                  tc.tile_pool(name="ps_dsm", bufs=2, space="PSUM"),
                  tc.tile_pool(name="kv", bufs=2),
                  tc.tile_pool(name="kv_v", bufs=2),
                  tc.tile_pool(name="ktsb", bufs=3))
                ps_kt, ps_sc, ps_dsm, kv_pool, kvv_pool, kt_pool = [
                    p.__enter__() for p in attn_loop]

                denom_flat = spool.tile([1, 32], F32, tag="denom_flat")
                nc.vector.memset(denom_flat[:], 0.0)
                CH = 4  # s-tiles per KV chunk (chunk = CH*128 seq positions)

                for b in range(B):
                    Tb = T[b]
                    if Tb == 0:
                        continue
                    n_ch = (Tb + CH - 1) // CH
                    sc_all = ps_sc.tile([128, 128], F32, tag="sc",
                                        name=f"sc_b{b}")
                    for ci in range(n_ch):
                        nt = min(CH, Tb - ci * CH)
                        k_sb = kv_pool.tile([128, CH * HPC * HD], F32,
                                            tag="k_sb", name=f"k_{b}_{ci}")
                        nc.sync.dma_start(
                            k_sb[:, 0:nt * HPC * HD]
                            .rearrange("p (i h d) -> p i (h d)", i=nt, h=HPC),
                            kc_d[b, ci * CH * 128:(ci * CH + nt) * 128, :, :]
                            .rearrange("(i p) h d -> p i (h d)", p=128))
                        for h in range(HPC):
                            for j in range(nt):
                                t = ci * CH + j
                                kt_ps = ps_kt.tile([128, 128], F32, tag="kt")
                                nc.tensor.matmul(
                                    kt_ps[:],
                                    k_sb[:, (j * HPC + h) * HD:
                                         (j * HPC + h + 1) * HD],
                                    ident[:, :], is_transpose=True,
                                    start=True, stop=True)
                                kt_sb = kt_pool.tile([128, 128], F32,
                                                     tag="kt_sb")
                                nc.vector.tensor_copy(kt_sb[:], kt_ps[:])
                                nc.tensor.matmul(
                                    sc_all[:, h * 32 + t:h * 32 + t + 1],
                                    kt_sb[:], qT[:, b * HPC + h:b * HPC + h + 1],
                                    start=True, stop=True)
                    exp_sb = exp_pool.tile([128, 128], F32, tag="exp",
                                           name=f"exp_b{b}")
                    er = exp_sb[:].rearrange("p (h t) -> p h t", h=HPC)
                    sr = sc_all[:].rearrange("p (h t) -> p h t", h=HPC)
                    if Tb > 1:
                        nc.scalar.activation(er[:, :, 0:Tb - 1],
                                             sr[:, :, 0:Tb - 1], AF.Exp,
                                             bias=zbias[:, :])
                    nc.scalar.activation(er[:, :, Tb - 1:Tb],
                                         sr[:, :, Tb - 1:Tb], AF.Exp,
                                         bias=maskT[:, b:b + 1])
                    dsm = ps_dsm.tile([1, 128], F32, tag="dsm",
                                      name=f"dsm_b{b}")
                    nc.tensor.matmul(
                        dsm[0:1, 0:HPC * Tb], ones_col[:],
                        er[:, :, 0:Tb], start=True, stop=True)
                    nc.vector.reduce_sum(
                        out=denom_flat[0:1, b * HPC:(b + 1) * HPC],
                        in_=dsm[0:1, 0:HPC * Tb]
                        .rearrange("o (h t) -> o h t", h=HPC),
                        axis=AX)
                    for ci in range(n_ch):
                        nt = min(CH, Tb - ci * CH)
                        v_sb = kvv_pool.tile([128, CH * HPC * HD], F32,
                                             tag="v_sbt", name=f"v_{b}_{ci}")
                        nc.sync.dma_start(
                            v_sb[:, 0:nt * HPC * HD]
                            .rearrange("p (i h d) -> p i (h d)", i=nt, h=HPC),
                            vc_d[b, ci * CH * 128:(ci * CH + nt) * 128, :, :]
                            .rearrange("(i p) h d -> p i (h d)", p=128))
                        for h in range(HPC):
                            for j in range(nt):
                                t = ci * CH + j
                                nc.tensor.matmul(
                                    attn_ps[:, b * HPC + h:b * HPC + h + 1],
                                    v_sb[:, (j * HPC + h) * HD:
                                         (j * HPC + h + 1) * HD],
                                    exp_sb[:, h * 32 + t:h * 32 + t + 1],
                                    start=False, stop=(t == Tb - 1))

                for p in reversed(attn_loop):
                    p.__exit__(None, None, None)

                # new-token contribution + denominators + normalize
                with tc.tile_pool(name="ps_attn2", bufs=1,
                                  space="PSUM") as ps_attn2:
                    attn_new_ps = ps_attn2.tile([128, 32], F32, tag="attn_new")
                    for h in range(HPC):
                        nc.tensor.matmul(
                            attn_new_ps[:].rearrange(
                                "p (b h) -> p b h", h=HPC)[:, :, h],
                            vw[:, h * HD:(h + 1) * HD], ident[0:B, 0:B],
                            is_transpose=True, start=True, stop=True)
                    enew_dram = dram.tile([B, HPC], F32, tag="enew_d")
                    nc.sync.dma_start(enew_dram[:], e_new[:])
                    enew_flat = spool.tile([1, 32], F32, tag="enew_flat")
                    nc.sync.dma_start(
                        enew_flat[:],
                        enew_dram[:].rearrange("b h -> () (b h)"))
                    nc.vector.tensor_tensor(out=denom_flat[:],
                                            in0=denom_flat[:],
                                            in1=enew_flat[:], op=OP.add)
                    recip_flat = spool.tile([1, 32], F32, tag="recip_flat")
                    nc.vector.reciprocal(recip_flat[:], denom_flat[:])
                    bcast_ps = ps_attn2.tile([128, 32], F32, tag="bcast")
                    nc.tensor.matmul(bcast_ps[:], ones_row[:], recip_flat[:],
                                     start=True, stop=True)
                    attn_tot = spool.tile([128, 32], F32, tag="attn_tot")
                    nc.vector.tensor_copy(attn_tot[:], attn_ps[:])
                    nc.vector.tensor_tensor(out=attn_tot[:], in0=attn_tot[:],
                                            in1=attn_new_ps[:], op=OP.add)
                    attn_n = spool.tile([128, 32], MDT, tag="attn_n")
                    nc.vector.tensor_tensor(out=attn_n[:], in0=attn_tot[:],
                                            in1=bcast_ps[:], op=OP.mult)

            # ============ phase 3: dense + AR1 ============
            ar1_in = dram.tile([B, HID], F32, tag="ar1_in")
            ar1_out = dram.tile([B, HID], F32, tag="ar1_out")
            attn_nr = attn_n[:].rearrange("p (b h) -> p b h", h=HPC)
            with (tc.tile_pool(name="ps_o8a", bufs=8, space="PSUM") as o8a,
                  tc.tile_pool(name="wdp", bufs=2) as wdp):
                dps = [o8a.tile([B, 512], F32, tag="o8", name=f"dps{n}")
                       for n in range(8)]
                for h in range(HPC):
                    wd_sb = wdp.tile([128, HID], MDT, tag="wd")
                    nc.sync.dma_start(wd_sb[:],
                                      g(wd_d[h * 128:(h + 1) * 128, :]))
                    for n in range(8):
                        nc.tensor.matmul(
                            dps[n][:], attn_nr[:, :, h],
                            wd_sb[:, n * 512:(n + 1) * 512],
                            start=(h == 0), stop=(h == HPC - 1))
                gout = spool.tile([B, HID], F32, tag="gemmout")
                for n in range(8):
                    nc.vector.tensor_copy(gout[:, n * 512:(n + 1) * 512],
                                          dps[n][:])
                nc.sync.dma_start(ar1_in[:], gout[:])
            nc.gpsimd.collective_compute(
                "AllReduce", mybir.AluOpType.add,
                replica_groups=[list(range(NC_))],
                ins=[ar1_in.opt()], outs=[ar1_out.opt()])

            x2 = spool.tile([B, HID], F32, tag="x2")
            ar_sb = spool.tile([B, HID], F32, tag="ar_sb")
            nc.sync.dma_start(ar_sb[:], ar1_out[:])
            nc.vector.tensor_tensor(out=x2[:], in0=hidden[:], in1=ar_sb[:],
                                    op=OP.add)

            # ============ phase 4: LN2 + FFN + AR2 ============
            hg = spool.tile([B, F1PC // 2], F32, tag="hg")
            with (tc.tile_pool(name="ps_tp2", bufs=2, space="PSUM") as tp2,
                  tc.tile_pool(name="ps_o8b", bufs=4, space="PSUM") as o8b,
                  tc.tile_pool(name="wf1p", bufs=3) as wf1p):
                xc2 = ln_stats(x2)
                x2T = ln_transpose(xc2, ln2wT, ln2bT, tp2, "x2T")
                for pair in range(4):
                    hp = [o8b.tile([B, 512], F32, tag="o8",
                                   name=f"hp{pair}_{n}") for n in range(2)]
                    for ki in range(32):
                        w_sb = wf1p.tile([128, 1024], MDT, tag="wf1c")
                        nc.sync.dma_start(
                            w_sb[:],
                            g(wf1_d[ki * 128:(ki + 1) * 128,
                                    pair * 1024:(pair + 1) * 1024]))
                        for n in range(2):
                            nc.tensor.matmul(
                                hp[n][:], x2T[:, ki * B:(ki + 1) * B],
                                w_sb[:, n * 512:(n + 1) * 512],
                                start=(ki == 0), stop=(ki == 31))
                    sil = spool.tile([B, 512], F32, tag="sil")
                    nc.scalar.activation(sil[:], hp[0][:], AF.Silu,
                                         bias=zbias[0:B, :])
                    nc.vector.tensor_tensor(
                        out=hg[:, pair * 512:(pair + 1) * 512],
                        in0=sil[:], in1=hp[1][:], op=OP.mult)

                hT = spool.tile([128, 16 * B], MDT, tag="hT")
                tpt = tp2.tile([128, 16 * B], F32, tag="tp")
                for i in range(16):
                    nc.tensor.matmul(
                        tpt[:, i * B:(i + 1) * B],
                        hg[:, i * 128:(i + 1) * 128], ident[0:B, 0:B],
                        is_transpose=True, start=True, stop=True)
                nc.vector.tensor_copy(hT[:], tpt[:])

            ar2_in = dram.tile([B, HID], F32, tag="ar2_in")
            ar2_out = dram.tile([B, HID], F32, tag="ar2_out")
            with (tc.tile_pool(name="ps_o8c", bufs=8, space="PSUM") as o8c,
                  tc.tile_pool(name="wf2p", bufs=2) as wf2p):
                fps = [o8c.tile([B, 512], F32, tag="o8", name=f"fps{n}")
                       for n in range(8)]
                for ki in range(16):
                    w_sb = wf2p.tile([128, HID], MDT, tag="wf2c")
                    nc.sync.dma_start(w_sb[:],
                                      g(wf2_d[ki * 128:(ki + 1) * 128, :]))
                    for n in range(8):
                        nc.tensor.matmul(
                            fps[n][:], hT[:, ki * B:(ki + 1) * B],
                            w_sb[:, n * 512:(n + 1) * 512],
                            start=(ki == 0), stop=(ki == 15))
                gout2 = spool.tile([B, HID], F32, tag="gemmout")
                for n in range(8):
                    nc.vector.tensor_copy(gout2[:, n * 512:(n + 1) * 512],
                                          fps[n][:])
                nc.sync.dma_start(ar2_in[:], gout2[:])
            nc.gpsimd.collective_compute(
                "AllReduce", mybir.AluOpType.add,
                replica_groups=[list(range(NC_))],
                ins=[ar2_in.opt()], outs=[ar2_out.opt()])

            ar2_sb = spool.tile([B, HID], F32, tag="ar2_sb")
            nc.sync.dma_start(ar2_sb[:], ar2_out[:])
            out_sb = spool.tile([B, HID], F32, tag="out_sb")
            nc.vector.tensor_tensor(out=out_sb[:], in0=x2[:], in1=ar2_sb[:],
                                    op=OP.add)
            nc.sync.dma_start(out_d[:], out_sb[:])

    nc.compile()
    return nc


def kernel(**inputs):
    from concourse.bass_utils import run_bass_kernel_spmd
    in_maps, T = _shard_inputs(inputs)
    nc = _build_program(T)
    trace = os.environ.get("KERNEL_TRACE", "0") == "1"
    tdir = os.environ.get("KERNEL_TRACE_DIR") or None
    res = run_bass_kernel_spmd(nc, in_maps, list(range(NC_)), trace=trace,
                               tmpdir=tdir)
    if trace and res.exec_time_ns is not None:
        print(f"HW exec time: {res.exec_time_ns} ns")
        print(f"mean exec time: {res.mean_exec_time_ns} ns "
              f"(max core {res.max_exec_time_core_id})")
        kernel.last_exec_time_ns = res.exec_time_ns
    return res.results[0]["out"]


if __name__ == "__main__":
    data = np.load("/tmp/ref_inputs.npz")
    inputs = {k: data[k] for k in data.files}
    ref = np.load("/tmp/ref_out.npy")
    out = kernel(**inputs)
    err = np.abs(out - ref).max()
    rel = err / np.abs(ref).max()
    print(f"absmax err: {err:.3e}  rel: {rel:.3e}")


# revision 16
# speedup vs baseline: 1.1964x; 1.1860x over previous
"""Trainium2 Bass kernel for nn_NVGPTDecoderLayer (single-token decode layer).

Tensor-parallel over 8 NeuronCores, vLLM style:
  - qkv column-parallel (4 heads/core), dense row-parallel
  - fc1 column-parallel, fc2 row-parallel (SwiGLU pairs interleaved host-side)
  - KV cache sharded along head dim; layernorm replicated
  - AllReduce after dense and after fc2

Attention tile counts are baked from `positions` at trace time (the program is
rebuilt per call, so this stays correct for whatever inputs are given).
"""
import os
import numpy as np

B, S, HID = 8, 4096, 4096
NH, HD, ROT = 32, 128, 64
FFN = 16384
EPS = 1e-5
SCALE = HD ** -0.5
NC_ = 8
HPC = NH // NC_          # 4 heads per core
QPC = HPC * HD           # 512
F1PC = 2 * FFN // NC_    # 4096 fc1 cols per core
F2PC = FFN // NC_        # 2048 fc2 rows per core


def _host_prep(positions):
    pos = np.asarray(positions).astype(np.int64)
    inv_freq = 1.0 / (10000.0 ** (np.arange(0, ROT, 2, dtype=np.float32) / ROT))
    ang = pos[:, None].astype(np.float32) * inv_freq[None, :]      # [8, 32]
    cos = np.cos(ang).astype(np.float32)
    sin = np.sin(ang).astype(np.float32)
    cosx_h = np.concatenate([cos, cos, np.ones((B, HD - ROT), np.float32)], 1)
    sinx_h = np.concatenate([-sin, sin, np.zeros((B, HD - ROT), np.float32)], 1)
    cosx = np.ascontiguousarray(np.tile(cosx_h, (1, HPC)))         # [8, 512]
    sinx = np.ascontiguousarray(np.tile(sinx_h, (1, HPC)))
    T = [int(np.ceil(p / 128)) if p > 0 else 0 for p in pos]
    maskT = np.zeros((128, B), np.float32)
    for b, p in enumerate(pos):
        t = T[b] - 1
        if t >= 0:
            jj = t * 128 + np.arange(128)
            maskT[:, b] = np.where(jj < p, 0.0, -1e30).astype(np.float32)
    return pos, cosx, sinx, T, maskT


def _shard_inputs(inputs):
    pos, cosx, sinx, T, maskT = _host_prep(inputs["positions"])
    h_np = np.ascontiguousarray(inputs["hidden_states"], dtype=np.float32)
    wq = inputs["w_qkv"]; wd = inputs["w_dense"]
    wf1 = inputs["w_fc1"]; wf2 = inputs["w_fc2"]
    lnT = {}
    for nm, w_, b_ in (("1", inputs["ln1_w"], inputs["ln1_b"]),
                       ("2", inputs["ln2_w"], inputs["ln2_b"])):
        lnT["w" + nm] = np.ascontiguousarray(
            (np.asarray(w_, dtype=np.float32) + 1.0).reshape(32, 128).T)
        lnT["b" + nm] = np.ascontiguousarray(
            np.asarray(b_, dtype=np.float32).reshape(32, 128).T)
    ident = np.eye(128, dtype=np.float32)
    in_maps = []
    for c in range(NC_):
        wq_c = np.ascontiguousarray(np.concatenate([
            wq[:, c * QPC:(c + 1) * QPC],
            wq[:, HID + c * QPC: HID + (c + 1) * QPC],
            wq[:, 2 * HID + c * QPC: 2 * HID + (c + 1) * QPC]], 1),
            dtype=np.float32)
        blocks = []
        for j in range(F1PC // 1024):
            o = c * (F1PC // 2) + j * 512
            blocks.append(wf1[:, o:o + 512])
            blocks.append(wf1[:, FFN + o: FFN + o + 512])
        wf1_c = np.ascontiguousarray(np.concatenate(blocks, 1), dtype=np.float32)
        m = {
            "hidden": h_np,
            "wqkv": wq_c,
            "wd": np.ascontiguousarray(wd[c * QPC:(c + 1) * QPC, :], dtype=np.float32),
            "wf1": wf1_c,
            "wf2": np.ascontiguousarray(wf2[c * F2PC:(c + 1) * F2PC, :], dtype=np.float32),
            "kc": np.ascontiguousarray(inputs["k_cache"][:, :, c * HPC:(c + 1) * HPC, :], dtype=np.float32),
            "vc": np.ascontiguousarray(inputs["v_cache"][:, :, c * HPC:(c + 1) * HPC, :], dtype=np.float32),
            "ln1wT": lnT["w1"], "ln1bT": lnT["b1"],
            "ln2wT": lnT["w2"], "ln2bT": lnT["b2"],
            "cosxq": (cosx * SCALE).astype(np.float32),
            "sinxq": (sinx * SCALE).astype(np.float32),
            "cosxk": cosx, "sinxk": sinx,
            "maskT": maskT, "ident": ident,
        }
        in_maps.append(m)
    return in_maps, T


def _build_program(T):
    import concourse.bass as bass  # noqa: F401
    import concourse.bacc as bacc
    import concourse.mybir as mybir
    import concourse.tile as tile

    F32 = mybir.dt.float32
    F32R = mybir.dt.float32r
    GEMM_DT = {"fp32": mybir.dt.float32, "fp32r": mybir.dt.float32r}[
        os.environ.get("KERNEL_GEMM_DT", "fp32r")]

    MDT = GEMM_DT

    def g(ap):
        return ap.bitcast(GEMM_DT)

    nc = bacc.Bacc("TRN2", target_bir_lowering=False, debug=False,
                   num_devices=NC_)

    def din(name, shape):
        return nc.dram_tensor(name, list(shape), F32, kind="ExternalInput").ap()

    hidden_d = din("hidden", (B, HID))
    wqkv_d = din("wqkv", (HID, 3 * QPC))
    wd_d = din("wd", (QPC, HID))
    wf1_d = din("wf1", (HID, F1PC))
    wf2_d = din("wf2", (F2PC, HID))
    kc_d = din("kc", (B, S, HPC, HD))
    vc_d = din("vc", (B, S, HPC, HD))
    ln1wT_d = din("ln1wT", (128, 32)); ln1bT_d = din("ln1bT", (128, 32))
    ln2wT_d = din("ln2wT", (128, 32)); ln2bT_d = din("ln2bT", (128, 32))
    cosxq_d = din("cosxq", (B, QPC)); sinxq_d = din("sinxq", (B, QPC))
    cosxk_d = din("cosxk", (B, QPC)); sinxk_d = din("sinxk", (B, QPC))
    maskT_d = din("maskT", (128, B))
    ident_d = din("ident", (128, 128))
    out_d = nc.dram_tensor("out", [B, HID], F32, kind="ExternalOutput").ap()

    AX = mybir.AxisListType.X
    OP = mybir.AluOpType
    AF = mybir.ActivationFunctionType

    with tile.TileContext(nc) as tc:
        with (tc.tile_pool(name="const", bufs=1) as cpool,
              tc.tile_pool(name="sbuf", bufs=1) as spool,
              tc.tile_pool(name="dram", bufs=1, space="DRAM") as dram):

            def cload(name, shape, src):
                t = cpool.tile(list(shape), F32, tag=name)
                nc.sync.dma_start(t[:], src[:])
                return t

            ident = cload("ident", (128, 128), ident_d)
            ln1wT = cload("ln1wT", (128, 32), ln1wT_d)
            ln1bT = cload("ln1bT", (128, 32), ln1bT_d)
            ln2wT = cload("ln2wT", (128, 32), ln2wT_d)
            ln2bT = cload("ln2bT", (128, 32), ln2bT_d)
            cosxq = cload("cosxq", (B, QPC), cosxq_d)
            sinxq = cload("sinxq", (B, QPC), sinxq_d)
            cosxk = cload("cosxk", (B, QPC), cosxk_d)
            sinxk = cload("sinxk", (B, QPC), sinxk_d)
            maskT = cload("maskT", (128, B), maskT_d)
            ones_col = cpool.tile([128, 1], F32, tag="ones_col")
            nc.vector.memset(ones_col[:], 1.0)
            ones_col_r = cpool.tile([128, 1], F32R, tag="ones_col_r")
            nc.vector.tensor_copy(ones_col_r[:], ones_col[:])
            ident_r = cpool.tile([128, 128], F32R, tag="ident_r")
            nc.sync.dma_start(ident_r[:], ident_d[:].bitcast(F32R))
            ones_row = cpool.tile([1, 128], F32, tag="ones_row")
            nc.vector.memset(ones_row[:], 1.0)
            zbias = cpool.tile([128, 1], F32, tag="zbias")
            nc.vector.memset(zbias[:], 0.0)
            ebias = cpool.tile([128, 1], F32, tag="ebias")
            nc.vector.memset(ebias[:], EPS)

            hidden = spool.tile([B, HID], F32, tag="hidden")
            nc.sync.dma_start(hidden[:], hidden_d[:])

            def ln_stats(x_sb):
                st = spool.tile([B, HID], F32, tag="lnst")
                ssum = spool.tile([B, 1], F32, tag="ln_sum")
                sqs = spool.tile([B, 1], F32, tag="ln_sqs")
                nc.vector.reduce_sum(out=ssum[:], in_=x_sb[:], axis=AX)
                nc.scalar.activation(st[:], x_sb[:], AF.Square,
                                     bias=zbias[0:B, :], accum_out=sqs[:])
                nmu = spool.tile([B, 1], F32, tag="ln_nmu")
                nc.vector.tensor_scalar_mul(nmu[:], ssum[:], -1.0 / HID)
                ex2 = spool.tile([B, 1], F32, tag="ln_ex2")
                nc.vector.tensor_scalar_mul(ex2[:], sqs[:], 1.0 / HID)
                mu2 = spool.tile([B, 1], F32, tag="ln_mu2")
                nc.vector.tensor_tensor(out=mu2[:], in0=nmu[:], in1=nmu[:],
                                        op=OP.mult)
                var = spool.tile([B, 1], F32, tag="ln_var")
                nc.vector.tensor_tensor(out=var[:], in0=ex2[:], in1=mu2[:],
                                        op=OP.subtract)
                sd = spool.tile([B, 1], F32, tag="ln_sd")
                nc.scalar.activation(sd[:], var[:], AF.Sqrt, bias=ebias[0:B, :])
                rstd = spool.tile([B, 1], F32, tag="ln_rstd")
                nc.vector.reciprocal(rstd[:], sd[:])
                xc = spool.tile([B, HID], F32, tag="ln_xc")
                nc.vector.tensor_scalar(out=xc[:], in0=x_sb[:],
                                        scalar1=nmu[:], scalar2=rstd[:],
                                        op0=OP.add, op1=OP.mult)
                return xc

            def ln_transpose(xc, wT, bT, tpool, out_tag):
                xT = spool.tile([128, 32 * B], MDT, tag=out_tag)
                for half in range(2):
                    tp = tpool.tile([128, 16 * B], F32, tag="tp")
                    for i2 in range(16):
                        i = half * 16 + i2
                        nc.tensor.matmul(
                            tp[:, i2 * B:(i2 + 1) * B],
                            xc[:, i * 128:(i + 1) * 128],
                            ident[0:B, 0:B], is_transpose=True,
                            start=True, stop=True)
                    for i2 in range(16):
                        i = half * 16 + i2
                        nc.vector.tensor_scalar(
                            out=xT[:, i * B:(i + 1) * B],
                            in0=tp[:, i2 * B:(i2 + 1) * B],
                            scalar1=wT[:, i:i + 1], scalar2=bT[:, i:i + 1],
                            op0=OP.mult, op1=OP.add)
                return xT

            def rope(ps, cosx_t, sinx_t, tag):
                r = spool.tile([B, QPC], F32, tag=tag)
                nc.vector.tensor_tensor(out=r[:], in0=ps[:], in1=cosx_t[:],
                                        op=OP.mult)
                t2 = spool.tile([B, HPC * ROT], F32, tag=tag + "_t2")
                ps_r = ps[:].rearrange("b (h r) -> b h r", h=HPC)
                sx_r = sinx_t[:].rearrange("b (h r) -> b h r", h=HPC)
                t2_r = t2[:].rearrange("b (h r) -> b h r", h=HPC)
                nc.vector.tensor_tensor(out=t2_r[:, :, 0:32],
                                        in0=ps_r[:, :, 32:64],
                                        in1=sx_r[:, :, 0:32], op=OP.mult)
                nc.vector.tensor_tensor(out=t2_r[:, :, 32:64],
                                        in0=ps_r[:, :, 0:32],
                                        in1=sx_r[:, :, 32:64], op=OP.mult)
                r_r = r[:].rearrange("b (h r) -> b h r", h=HPC)
                nc.vector.tensor_tensor(out=r_r[:, :, 0:ROT],
                                        in0=r_r[:, :, 0:ROT], in1=t2_r[:],
                                        op=OP.add)
                return r

            # ============ phase 1: LN1 + QKV + RoPE + qT ============
            with (tc.tile_pool(name="ps_tp1", bufs=2, space="PSUM") as tp1,
                  tc.tile_pool(name="ps_qkv", bufs=1, space="PSUM") as ps_qkv,
                  tc.tile_pool(name="wq", bufs=2) as wq_pool):
                xc1 = ln_stats(hidden)
                x1T = ln_transpose(xc1, ln1wT, ln1bT, tp1, "x1T")

                qkv_ps = [ps_qkv.tile([B, 512], F32, tag=f"qkv{n}",
                                      name=f"qkv_ps{n}") for n in range(3)]
                for ki in range(32):
                    wq_sb = wq_pool.tile([128, 3 * QPC], MDT, tag="wq")
                    nc.scalar.dma_start(wq_sb[:],
                                        g(wqkv_d[ki * 128:(ki + 1) * 128, :]))
                    for n in range(3):
                        nc.tensor.matmul(
                            qkv_ps[n][:], x1T[:, ki * B:(ki + 1) * B],
                            wq_sb[:, n * 512:(n + 1) * 512],
                            start=(ki == 0), stop=(ki == 31))

                qr = rope(qkv_ps[0], cosxq, sinxq, "qr")
                kr = rope(qkv_ps[1], cosxk, sinxk, "kr")
                v_sb = spool.tile([B, QPC], F32, tag="v_sb")
                nc.vector.tensor_copy(v_sb[:], qkv_ps[2][:])

                qT_ps = tp1.tile([128, 32], F32, tag="tp")
                for h in range(HPC):
                    nc.tensor.matmul(
                        qT_ps[:].rearrange("p (b h) -> p b h", h=HPC)[:, :, h],
                        qr[:, h * HD:(h + 1) * HD], ident[0:B, 0:B],
                        is_transpose=True, start=True, stop=True)
                qT = spool.tile([128, 64], F32R, tag="qT")
                nc.vector.memset(qT[:].bitcast(F32), 0.0)
                nc.vector.tensor_copy(
                    qT[:].rearrange("p (m l) -> p m l", l=2)[:, :, 0],
                    qT_ps[:])

                tmp_qk = spool.tile([B, QPC], F32, tag="tmp_qk")
                nc.vector.tensor_tensor(out=tmp_qk[:], in0=qr[:], in1=kr[:],
                                        op=OP.mult)
                s_new = spool.tile([B, HPC], F32, tag="s_new")
                nc.vector.reduce_sum(
                    out=s_new[:],
                    in_=tmp_qk[:].rearrange("b (h d) -> b h d", h=HPC),
                    axis=AX)
                e_new = spool.tile([B, HPC], F32, tag="e_new")
                nc.scalar.activation(e_new[:], s_new[:], AF.Exp, bias=zbias[0:B, :])

                vw = spool.tile([B, QPC], F32, tag="vw")
                for h in range(HPC):
                    nc.vector.tensor_scalar_mul(
                        vw[:, h * HD:(h + 1) * HD],
                        v_sb[:, h * HD:(h + 1) * HD], e_new[:, h:h + 1])

            # ============ phase 2: attention ============
            with (tc.tile_pool(name="ps_attn", bufs=1, space="PSUM") as ps_attn,
                  tc.tile_pool(name="exp", bufs=4) as exp_pool):
                attn_ps = ps_attn.tile([128, 64], F32, tag="attn")
                nc.vector.memset(attn_ps[:], 0.0)

                attn_loop = (
                  tc.tile_pool(name="ps_kt", bufs=2, space="PSUM"),
                  tc.tile_pool(name="ps_sc", bufs=2, space="PSUM"),
                  tc.tile_pool(name="ps_dsm", bufs=2, space="PSUM"),
                  tc.tile_pool(name="kv", bufs=2),
                  tc.tile_pool(name="kv_v", bufs=2),
                  tc.tile_pool(name="ktsb", bufs=3))
                ps_kt, ps_sc, ps_dsm, kv_pool, kvv_pool, kt_pool = [
                    p.__enter__() for p in attn_loop]

                denom_flat = spool.tile([1, 32], F32, tag="denom_flat")
                nc.vector.memset(denom_flat[:], 0.0)
                CH = 4  # s-tiles per KV chunk (chunk = CH*128 seq positions)

                for b in range(B):
                    Tb = T[b]
                    if Tb == 0:
                        continue
                    n_ch = (Tb + CH - 1) // CH
                    Tev = Tb + (Tb & 1)
                    sc_all = ps_sc.tile([128, 256], F32, tag="sc",
                                        name=f"sc_b{b}")
                    exp_sb = exp_pool.tile([128, 256], F32R, tag="exp",
                                           name=f"exp_b{b}")
                    nc.vector.memset(exp_sb[:].bitcast(F32), 0.0)
                    for ci in range(n_ch):
                        nt = min(CH, Tb - ci * CH)
                        k_sb = kv_pool.tile([128, CH * HPC * HD], F32R,
                                            tag="k_sb", name=f"k_{b}_{ci}")
                        nc.sync.dma_start(
                            k_sb[:, 0:nt * HPC * HD]
                            .rearrange("p (i h d) -> p i (h d)", i=nt, h=HPC),
                            kc_d[b, ci * CH * 128:(ci * CH + nt) * 128, :, :]
                            .bitcast(F32R)
                            .rearrange("(i p) h d -> p i (h d)", p=128))
                        for h in range(HPC):
                            for j in range(nt):
                                t = ci * CH + j
                                kt_ps = ps_kt.tile([128, 128], F32, tag="kt")
                                nc.tensor.matmul(
                                    kt_ps[:].bitcast(F32R),
                                    k_sb[:, (j * HPC + h) * HD:
                                         (j * HPC + h + 1) * HD],
                                    ident_r[:, :], is_transpose=True,
                                    start=True, stop=True)
                                kt_sb = kt_pool.tile([128, 128], F32R,
                                                     tag="kt_sb")
                                nc.vector.tensor_copy(kt_sb[:], kt_ps[:])
                                c = h * 64 + 2 * t
                                m = b * HPC + h
                                nc.tensor.matmul(
                                    sc_all[:, c:c + 2], kt_sb[:],
                                    qT[:, 2 * m:2 * m + 2],
                                    start=True, stop=True)
                    er = exp_sb[:].rearrange("p (h t l) -> p h t l",
                                             h=HPC, l=2)
                    sr = sc_all[:].rearrange("p (h t l) -> p h t l",
                                             h=HPC, l=2)
                    if Tb > 1:
                        nc.scalar.activation(er[:, :, 0:Tb - 1, 0],
                                             sr[:, :, 0:Tb - 1, 0], AF.Exp,
                                             bias=zbias[:, :])
                    nc.scalar.activation(er[:, :, Tb - 1:Tb, 0],
                                         sr[:, :, Tb - 1:Tb, 0], AF.Exp,
                                         bias=maskT[:, b:b + 1])
                    dsm = ps_dsm.tile([1, 128], F32, tag="dsm",
                                      name=f"dsm_b{b}")
                    nc.tensor.matmul(
                        dsm[0:1, 0:HPC * Tev], ones_col_r[:],
                        er[:, :, 0:Tev, 0], start=True, stop=True)
                    nc.vector.reduce_sum(
                        out=denom_flat[0:1, b * HPC:(b + 1) * HPC],
                        in_=dsm[0:1, 0:HPC * Tev]
                        .rearrange("o (h t) -> o h t", h=HPC),
                        axis=AX)
                    for ci in range(n_ch):
                        nt = min(CH, Tb - ci * CH)
                        v_sb = kvv_pool.tile([128, CH * HPC * HD], F32R,
                                             tag="v_sbt", name=f"v_{b}_{ci}")
                        nc.scalar.dma_start(
                            v_sb[:, 0:nt * HPC * HD]
                            .rearrange("p (i h d) -> p i (h d)", i=nt, h=HPC),
                            vc_d[b, ci * CH * 128:(ci * CH + nt) * 128, :, :]
                            .bitcast(F32R)
                            .rearrange("(i p) h d -> p i (h d)", p=128))
                        for h in range(HPC):
                            for j in range(nt):
                                t = ci * CH + j
                                c = h * 64 + 2 * t
                                m = b * HPC + h
                                nc.tensor.matmul(
                                    attn_ps[:, 2 * m:2 * m + 2],
                                    v_sb[:, (j * HPC + h) * HD:
                                         (j * HPC + h + 1) * HD],
                                    exp_sb[:, c:c + 2],
                                    start=False, stop=(t == Tb - 1))

                for p in reversed(attn_loop):
                    p.__exit__(None, None, None)

                # new-token contribution + denominators + normalize
                with tc.tile_pool(name="ps_attn2", bufs=1,
                                  space="PSUM") as ps_attn2:
                    attn_new_ps = ps_attn2.tile([128, 32], F32, tag="attn_new")
                    for h in range(HPC):
                        nc.tensor.matmul(
                            attn_new_ps[:].rearrange(
                                "p (b h) -> p b h", h=HPC)[:, :, h],
                            vw[:, h * HD:(h + 1) * HD], ident[0:B, 0:B],
                            is_transpose=True, start=True, stop=True)
                    enew_dram = dram.tile([B, HPC], F32, tag="enew_d")
                    nc.sync.dma_start(enew_dram[:], e_new[:])
                    enew_flat = spool.tile([1, 32], F32, tag="enew_flat")
                    nc.sync.dma_start(
                        enew_flat[:],
                        enew_dram[:].rearrange("b h -> () (b h)"))
                    nc.vector.tensor_tensor(out=denom_flat[:],
                                            in0=denom_flat[:],
                                            in1=enew_flat[:], op=OP.add)
                    recip_flat = spool.tile([1, 32], F32, tag="recip_flat")
                    nc.vector.reciprocal(recip_flat[:], denom_flat[:])
                    bcast_ps = ps_attn2.tile([128, 32], F32, tag="bcast")
                    nc.tensor.matmul(bcast_ps[:], ones_row[:], recip_flat[:],
                                     start=True, stop=True)
                    attn_tot = spool.tile([128, 32], F32, tag="attn_tot")
                    nc.vector.tensor_copy(
                        attn_tot[:],
                        attn_ps[:].rearrange("p (m l) -> p m l", l=2)[:, :, 0])
                    nc.vector.tensor_tensor(out=attn_tot[:], in0=attn_tot[:],
                                            in1=attn_new_ps[:], op=OP.add)
                    attn_n = spool.tile([128, 32], MDT, tag="attn_n")
                    nc.vector.tensor_tensor(out=attn_n[:], in0=attn_tot[:],
                                            in1=bcast_ps[:], op=OP.mult)

            # ============ phase 3: dense + AR1 ============
            ar1_in = dram.tile([B, HID], F32, tag="ar1_in")
            ar1_out = dram.tile([B, HID], F32, tag="ar1_out")
            attn_nr = attn_n[:].rearrange("p (b h) -> p b h", h=HPC)
            with (tc.tile_pool(name="ps_o8a", bufs=8, space="PSUM") as o8a,
                  tc.tile_pool(name="wdp", bufs=2) as wdp):
                dps = [o8a.tile([B, 512], F32, tag="o8", name=f"dps{n}")
                       for n in range(8)]
                for h in range(HPC):
                    wd_sb = wdp.tile([128, HID], MDT, tag="wd")
                    nc.sync.dma_start(wd_sb[:],
                                      g(wd_d[h * 128:(h + 1) * 128, :]))
                    for n in range(8):
                        nc.tensor.matmul(
                            dps[n][:], attn_nr[:, :, h],
                            wd_sb[:, n * 512:(n + 1) * 512],
                            start=(h == 0), stop=(h == HPC - 1))
                gout = spool.tile([B, HID], F32, tag="gemmout")
                for n in range(8):
                    nc.vector.tensor_copy(gout[:, n * 512:(n + 1) * 512],
                                          dps[n][:])
                nc.sync.dma_start(ar1_in[:], gout[:])
            nc.gpsimd.collective_compute(
                "AllReduce", mybir.AluOpType.add,
                replica_groups=[list(range(NC_))],
                ins=[ar1_in.opt()], outs=[ar1_out.opt()])

            x2 = spool.tile([B, HID], F32, tag="x2")
            ar_sb = spool.tile([B, HID], F32, tag="ar_sb")
            nc.sync.dma_start(ar_sb[:], ar1_out[:])
            nc.vector.tensor_tensor(out=x2[:], in0=hidden[:], in1=ar_sb[:],
                                    op=OP.add)

            # ============ phase 4: LN2 + FFN + AR2 ============
            hg = spool.tile([B, F1PC // 2], F32, tag="hg")
            with (tc.tile_pool(name="ps_tp2", bufs=2, space="PSUM") as tp2,
                  tc.tile_pool(name="ps_o8b", bufs=4, space="PSUM") as o8b,
                  tc.tile_pool(name="wf1p", bufs=3) as wf1p):
                xc2 = ln_stats(x2)
                x2T = ln_transpose(xc2, ln2wT, ln2bT, tp2, "x2T")
                for pair in range(4):
                    hp = [o8b.tile([B, 512], F32, tag="o8",
                                   name=f"hp{pair}_{n}") for n in range(2)]
                    for ki in range(32):
                        w_sb = wf1p.tile([128, 1024], MDT, tag="wf1c")
                        eng = nc.sync if ki % 2 == 0 else nc.scalar
                        eng.dma_start(
                            w_sb[:],
                            g(wf1_d[ki * 128:(ki + 1) * 128,
                                    pair * 1024:(pair + 1) * 1024]))
                        for n in range(2):
                            nc.tensor.matmul(
                                hp[n][:], x2T[:, ki * B:(ki + 1) * B],
                                w_sb[:, n * 512:(n + 1) * 512],
                                start=(ki == 0), stop=(ki == 31))
                    sil = spool.tile([B, 512], F32, tag="sil")
                    nc.scalar.activation(sil[:], hp[0][:], AF.Silu,
                                         bias=zbias[0:B, :])
                    nc.vector.tensor_tensor(
                        out=hg[:, pair * 512:(pair + 1) * 512],
                        in0=sil[:], in1=hp[1][:], op=OP.mult)

                hT = spool.tile([128, 16 * B], MDT, tag="hT")
                tpt = tp2.tile([128, 16 * B], F32, tag="tp")
                for i in range(16):
                    nc.tensor.matmul(
                        tpt[:, i * B:(i + 1) * B],
                        hg[:, i * 128:(i + 1) * 128], ident[0:B, 0:B],
                        is_transpose=True, start=True, stop=True)
                nc.vector.tensor_copy(hT[:], tpt[:])

            ar2_in = dram.tile([B, HID], F32, tag="ar2_in")
            ar2_out = dram.tile([B, HID], F32, tag="ar2_out")
            with (tc.tile_pool(name="ps_o8c", bufs=8, space="PSUM") as o8c,
                  tc.tile_pool(name="wf2p", bufs=2) as wf2p):
                fps = [o8c.tile([B, 512], F32, tag="o8", name=f"fps{n}")
                       for n in range(8)]
                for ki in range(16):
                    w_sb = wf2p.tile([128, HID], MDT, tag="wf2c")
                    nc.scalar.dma_start(w_sb[:],
                                        g(wf2_d[ki * 128:(ki + 1) * 128, :]))
                    for n in range(8):
                        nc.tensor.matmul(
                            fps[n][:], hT[:, ki * B:(ki + 1) * B],
                            w_sb[:, n * 512:(n + 1) * 512],
                            start=(ki == 0), stop=(ki == 15))
                gout2 = spool.tile([B, HID], F32, tag="gemmout")
                for n in range(8):
                    nc.vector.tensor_copy(gout2[:, n * 512:(n + 1) * 512],
                                          fps[n][:])
                nc.sync.dma_start(ar2_in[:], gout2[:])
            nc.gpsimd.collective_compute(
                "AllReduce", mybir.AluOpType.add,
                replica_groups=[list(range(NC_))],
                ins=[ar2_in.opt()], outs=[ar2_out.opt()])

            ar2_sb = spool.tile([B, HID], F32, tag="ar2_sb")
            nc.sync.dma_start(ar2_sb[:], ar2_out[:])
            out_sb = spool.tile([B, HID], F32, tag="out_sb")
            nc.vector.tensor_tensor(out=out_sb[:], in0=x2[:], in1=ar2_sb[:],
                                    op=OP.add)
            nc.sync.dma_start(out_d[:], out_sb[:])

    nc.compile()
    return nc


def kernel(**inputs):
    from concourse.bass_utils import run_bass_kernel_spmd
    in_maps, T = _shard_inputs(inputs)
    nc = _build_program(T)
    trace = os.environ.get("KERNEL_TRACE", "0") == "1"
    tdir = os.environ.get("KERNEL_TRACE_DIR") or None
    res = run_bass_kernel_spmd(nc, in_maps, list(range(NC_)), trace=trace,
                               tmpdir=tdir)
    if trace and res.exec_time_ns is not None:
        print(f"HW exec time: {res.exec_time_ns} ns")
        print(f"mean exec time: {res.mean_exec_time_ns} ns "
              f"(max core {res.max_exec_time_core_id})")
        kernel.last_exec_time_ns = res.exec_time_ns
    return res.results[0]["out"]


if __name__ == "__main__":
    data = np.load("/tmp/ref_inputs.npz")
    inputs = {k: data[k] for k in data.files}
    ref = np.load("/tmp/ref_out.npy")
    out = kernel(**inputs)
    err = np.abs(out - ref).max()
    rel = err / np.abs(ref).max()
    print(f"absmax err: {err:.3e}  rel: {rel:.3e}")


# revision 19
# speedup vs baseline: 1.2783x; 1.0685x over previous
"""Trainium2 Bass kernel for nn_NVGPTDecoderLayer (single-token decode layer).

Tensor-parallel over 8 NeuronCores, vLLM style:
  - qkv column-parallel (4 heads/core), dense row-parallel
  - fc1 column-parallel, fc2 row-parallel (SwiGLU pairs interleaved host-side)
  - KV cache sharded along head dim; layernorm replicated
  - AllReduce after dense and after fc2

Attention tile counts are baked from `positions` at trace time (the program is
rebuilt per call, so this stays correct for whatever inputs are given).
"""
import os
import numpy as np

B, S, HID = 8, 4096, 4096
NH, HD, ROT = 32, 128, 64
FFN = 16384
EPS = 1e-5
SCALE = HD ** -0.5
NC_ = 8
HPC = NH // NC_          # 4 heads per core
QPC = HPC * HD           # 512
F1PC = 2 * FFN // NC_    # 4096 fc1 cols per core
F2PC = FFN // NC_        # 2048 fc2 rows per core


def _host_prep(positions):
    pos = np.asarray(positions).astype(np.int64)
    inv_freq = 1.0 / (10000.0 ** (np.arange(0, ROT, 2, dtype=np.float32) / ROT))
    ang = pos[:, None].astype(np.float32) * inv_freq[None, :]      # [8, 32]
    cos = np.cos(ang).astype(np.float32)
    sin = np.sin(ang).astype(np.float32)
    cosx_h = np.concatenate([cos, cos, np.ones((B, HD - ROT), np.float32)], 1)
    sinx_h = np.concatenate([-sin, sin, np.zeros((B, HD - ROT), np.float32)], 1)
    cosx = np.ascontiguousarray(np.tile(cosx_h, (1, HPC)))         # [8, 512]
    sinx = np.ascontiguousarray(np.tile(sinx_h, (1, HPC)))
    T = [int(np.ceil(p / 128)) if p > 0 else 0 for p in pos]
    maskT = np.zeros((128, B), np.float32)
    for b, p in enumerate(pos):
        t = T[b] - 1
        if t >= 0:
            jj = t * 128 + np.arange(128)
            maskT[:, b] = np.where(jj < p, 0.0, -1e30).astype(np.float32)
    return pos, cosx, sinx, T, maskT


def _shard_inputs(inputs):
    pos, cosx, sinx, T, maskT = _host_prep(inputs["positions"])
    h_np = np.ascontiguousarray(inputs["hidden_states"], dtype=np.float32)
    wq = inputs["w_qkv"]; wd = inputs["w_dense"]
    wf1 = inputs["w_fc1"]; wf2 = inputs["w_fc2"]
    lnT = {}
    for nm, w_, b_ in (("1", inputs["ln1_w"], inputs["ln1_b"]),
                       ("2", inputs["ln2_w"], inputs["ln2_b"])):
        lnT["w" + nm] = np.ascontiguousarray(
            (np.asarray(w_, dtype=np.float32) + 1.0).reshape(32, 128).T)
        lnT["b" + nm] = np.ascontiguousarray(
            np.asarray(b_, dtype=np.float32).reshape(32, 128).T)
    ident = np.eye(128, dtype=np.float32)
    in_maps = []
    for c in range(NC_):
        wq_c = np.ascontiguousarray(np.concatenate([
            wq[:, c * QPC:(c + 1) * QPC],
            wq[:, HID + c * QPC: HID + (c + 1) * QPC],
            wq[:, 2 * HID + c * QPC: 2 * HID + (c + 1) * QPC]], 1),
            dtype=np.float32)
        blocks = []
        for j in range(F1PC // 1024):
            o = c * (F1PC // 2) + j * 512
            blocks.append(wf1[:, o:o + 512])
            blocks.append(wf1[:, FFN + o: FFN + o + 512])
        wf1_c = np.ascontiguousarray(np.concatenate(blocks, 1), dtype=np.float32)
        m = {
            "hidden": h_np,
            "wqkv": wq_c,
            "wd": np.ascontiguousarray(wd[c * QPC:(c + 1) * QPC, :], dtype=np.float32),
            "wf1": wf1_c,
            "wf2": np.ascontiguousarray(wf2[c * F2PC:(c + 1) * F2PC, :], dtype=np.float32),
            "kc": np.ascontiguousarray(inputs["k_cache"][:, :, c * HPC:(c + 1) * HPC, :], dtype=np.float32),
            "vc": np.ascontiguousarray(inputs["v_cache"][:, :, c * HPC:(c + 1) * HPC, :], dtype=np.float32),
            "ln1wT": lnT["w1"], "ln1bT": lnT["b1"],
            "ln2wT": lnT["w2"], "ln2bT": lnT["b2"],
            "cosxq": (cosx * SCALE).astype(np.float32),
            "sinxq": (sinx * SCALE).astype(np.float32),
            "cosxk": cosx, "sinxk": sinx,
            "maskT": maskT, "ident": ident,
        }
        in_maps.append(m)
    return in_maps, T


def _build_program(T):
    import concourse.bass as bass  # noqa: F401
    import concourse.bacc as bacc
    import concourse.mybir as mybir
    import concourse.tile as tile

    F32 = mybir.dt.float32
    F32R = mybir.dt.float32r
    GEMM_DT = {"fp32": mybir.dt.float32, "fp32r": mybir.dt.float32r}[
        os.environ.get("KERNEL_GEMM_DT", "fp32r")]

    MDT = GEMM_DT

    def g(ap):
        return ap.bitcast(GEMM_DT)

    nc = bacc.Bacc("TRN2", target_bir_lowering=False, debug=False,
                   num_devices=NC_)

    def din(name, shape):
        return nc.dram_tensor(name, list(shape), F32, kind="ExternalInput").ap()

    hidden_d = din("hidden", (B, HID))
    wqkv_d = din("wqkv", (HID, 3 * QPC))
    wd_d = din("wd", (QPC, HID))
    wf1_d = din("wf1", (HID, F1PC))
    wf2_d = din("wf2", (F2PC, HID))
    kc_d = din("kc", (B, S, HPC, HD))
    vc_d = din("vc", (B, S, HPC, HD))
    ln1wT_d = din("ln1wT", (128, 32)); ln1bT_d = din("ln1bT", (128, 32))
    ln2wT_d = din("ln2wT", (128, 32)); ln2bT_d = din("ln2bT", (128, 32))
    cosxq_d = din("cosxq", (B, QPC)); sinxq_d = din("sinxq", (B, QPC))
    cosxk_d = din("cosxk", (B, QPC)); sinxk_d = din("sinxk", (B, QPC))
    maskT_d = din("maskT", (128, B))
    ident_d = din("ident", (128, 128))
    out_d = nc.dram_tensor("out", [B, HID], F32, kind="ExternalOutput").ap()

    AX = mybir.AxisListType.X
    OP = mybir.AluOpType
    AF = mybir.ActivationFunctionType

    with tile.TileContext(nc) as tc:
        with (tc.tile_pool(name="const", bufs=1) as cpool,
              tc.tile_pool(name="sbuf", bufs=1) as spool,
              tc.tile_pool(name="dram", bufs=1, space="DRAM") as dram):

            def cload(name, shape, src):
                t = cpool.tile(list(shape), F32, tag=name)
                nc.sync.dma_start(t[:], src[:])
                return t

            ident = cload("ident", (128, 128), ident_d)
            ln1wT = cload("ln1wT", (128, 32), ln1wT_d)
            ln1bT = cload("ln1bT", (128, 32), ln1bT_d)
            ln2wT = cload("ln2wT", (128, 32), ln2wT_d)
            ln2bT = cload("ln2bT", (128, 32), ln2bT_d)
            cosxq = cload("cosxq", (B, QPC), cosxq_d)
            sinxq = cload("sinxq", (B, QPC), sinxq_d)
            cosxk = cload("cosxk", (B, QPC), cosxk_d)
            sinxk = cload("sinxk", (B, QPC), sinxk_d)
            maskT = cload("maskT", (128, B), maskT_d)
            ones_col = cpool.tile([128, 1], F32, tag="ones_col")
            nc.vector.memset(ones_col[:], 1.0)
            ones_col_r = cpool.tile([128, 1], F32R, tag="ones_col_r")
            nc.vector.tensor_copy(ones_col_r[:], ones_col[:])
            ident_r = cpool.tile([128, 128], F32R, tag="ident_r")
            nc.sync.dma_start(ident_r[:], ident_d[:].bitcast(F32R))
            ones_row = cpool.tile([1, 128], F32, tag="ones_row")
            nc.vector.memset(ones_row[:], 1.0)
            zbias = cpool.tile([128, 1], F32, tag="zbias")
            nc.vector.memset(zbias[:], 0.0)
            ebias = cpool.tile([128, 1], F32, tag="ebias")
            nc.vector.memset(ebias[:], EPS)

            hidden = spool.tile([B, HID], F32, tag="hidden")
            nc.sync.dma_start(hidden[:], hidden_d[:])

            def ln_stats(x_sb):
                xc = spool.tile([B, HID], F32, tag="ln_xc")
                ssum = spool.tile([B, 1], F32, tag="ln_sum")
                sqs = spool.tile([B, 1], F32, tag="ln_sqs")
                nc.vector.reduce_sum(out=ssum[:], in_=x_sb[:], axis=AX)
                nc.scalar.activation(xc[:], x_sb[:], AF.Square,
                                     bias=zbias[0:B, :], accum_out=sqs[:])
                nmu = spool.tile([B, 1], F32, tag="ln_nmu")
                nc.vector.tensor_scalar_mul(nmu[:], ssum[:], -1.0 / HID)
                ex2 = spool.tile([B, 1], F32, tag="ln_ex2")
                nc.vector.tensor_scalar_mul(ex2[:], sqs[:], 1.0 / HID)
                mu2 = spool.tile([B, 1], F32, tag="ln_mu2")
                nc.vector.tensor_tensor(out=mu2[:], in0=nmu[:], in1=nmu[:],
                                        op=OP.mult)
                var = spool.tile([B, 1], F32, tag="ln_var")
                nc.vector.tensor_tensor(out=var[:], in0=ex2[:], in1=mu2[:],
                                        op=OP.subtract)
                sd = spool.tile([B, 1], F32, tag="ln_sd")
                nc.scalar.activation(sd[:], var[:], AF.Sqrt, bias=ebias[0:B, :])
                rstd = spool.tile([B, 1], F32, tag="ln_rstd")
                nc.vector.reciprocal(rstd[:], sd[:])
                nc.vector.tensor_scalar(out=xc[:], in0=x_sb[:],
                                        scalar1=nmu[:], scalar2=rstd[:],
                                        op0=OP.add, op1=OP.mult)
                return xc

            def ln_transpose(xc, wT, bT, tpool, out_tag):
                xT = spool.tile([128, 32 * B], MDT, tag=out_tag)
                for half in range(2):
                    tp = tpool.tile([128, 16 * B], F32, tag="tp")
                    for i2 in range(16):
                        i = half * 16 + i2
                        nc.tensor.matmul(
                            tp[:, i2 * B:(i2 + 1) * B],
                            xc[:, i * 128:(i + 1) * 128],
                            ident[0:B, 0:B], is_transpose=True,
                            start=True, stop=True)
                    for i2 in range(16):
                        i = half * 16 + i2
                        nc.vector.tensor_scalar(
                            out=xT[:, i * B:(i + 1) * B],
                            in0=tp[:, i2 * B:(i2 + 1) * B],
                            scalar1=wT[:, i:i + 1], scalar2=bT[:, i:i + 1],
                            op0=OP.mult, op1=OP.add)
                return xT

            def rope(ps, cosx_t, sinx_t, tag):
                r = spool.tile([B, QPC], F32, tag=tag)
                nc.vector.tensor_tensor(out=r[:], in0=ps[:], in1=cosx_t[:],
                                        op=OP.mult)
                t2 = spool.tile([B, HPC * ROT], F32, tag=tag + "_t2")
                ps_r = ps[:].rearrange("b (h r) -> b h r", h=HPC)
                sx_r = sinx_t[:].rearrange("b (h r) -> b h r", h=HPC)
                t2_r = t2[:].rearrange("b (h r) -> b h r", h=HPC)
                nc.vector.tensor_tensor(out=t2_r[:, :, 0:32],
                                        in0=ps_r[:, :, 32:64],
                                        in1=sx_r[:, :, 0:32], op=OP.mult)
                nc.vector.tensor_tensor(out=t2_r[:, :, 32:64],
                                        in0=ps_r[:, :, 0:32],
                                        in1=sx_r[:, :, 32:64], op=OP.mult)
                r_r = r[:].rearrange("b (h r) -> b h r", h=HPC)
                nc.vector.tensor_tensor(out=r_r[:, :, 0:ROT],
                                        in0=r_r[:, :, 0:ROT], in1=t2_r[:],
                                        op=OP.add)
                return r

            # ============ phase 1: LN1 + QKV + RoPE + qT ============
            with (tc.tile_pool(name="ps_tp1", bufs=2, space="PSUM") as tp1,
                  tc.tile_pool(name="ps_qkv", bufs=1, space="PSUM") as ps_qkv,
                  tc.tile_pool(name="wq", bufs=2) as wq_pool):
                xc1 = ln_stats(hidden)
                x1T = ln_transpose(xc1, ln1wT, ln1bT, tp1, "x1T")

                qkv_ps = [ps_qkv.tile([B, 512], F32, tag=f"qkv{n}",
                                      name=f"qkv_ps{n}") for n in range(3)]
                for ki in range(32):
                    wq_sb = wq_pool.tile([128, 3 * QPC], MDT, tag="wq")
                    nc.scalar.dma_start(wq_sb[:],
                                        g(wqkv_d[ki * 128:(ki + 1) * 128, :]))
                    for n in range(3):
                        nc.tensor.matmul(
                            qkv_ps[n][:], x1T[:, ki * B:(ki + 1) * B],
                            wq_sb[:, n * 512:(n + 1) * 512],
                            start=(ki == 0), stop=(ki == 31))

                qr = rope(qkv_ps[0], cosxq, sinxq, "qr")
                kr = rope(qkv_ps[1], cosxk, sinxk, "kr")
                v_sb = spool.tile([B, QPC], F32, tag="v_sb")
                nc.vector.tensor_copy(v_sb[:], qkv_ps[2][:])

                qT_ps = tp1.tile([128, 32], F32, tag="tp")
                for h in range(HPC):
                    nc.tensor.matmul(
                        qT_ps[:].rearrange("p (b h) -> p b h", h=HPC)[:, :, h],
                        qr[:, h * HD:(h + 1) * HD], ident[0:B, 0:B],
                        is_transpose=True, start=True, stop=True)
                qT = spool.tile([128, 64], F32R, tag="qT")
                nc.vector.memset(qT[:].bitcast(F32), 0.0)
                nc.vector.tensor_copy(
                    qT[:].rearrange("p (m l) -> p m l", l=2)[:, :, 0],
                    qT_ps[:])

                tmp_qk = spool.tile([B, QPC], F32, tag="tmp_qk")
                nc.vector.tensor_tensor(out=tmp_qk[:], in0=qr[:], in1=kr[:],
                                        op=OP.mult)
                s_new = spool.tile([B, HPC], F32, tag="s_new")
                nc.vector.reduce_sum(
                    out=s_new[:],
                    in_=tmp_qk[:].rearrange("b (h d) -> b h d", h=HPC),
                    axis=AX)
                e_new = spool.tile([B, HPC], F32, tag="e_new")
                nc.scalar.activation(e_new[:], s_new[:], AF.Exp, bias=zbias[0:B, :])

                vw = spool.tile([B, QPC], F32, tag="vw")
                for h in range(HPC):
                    nc.vector.tensor_scalar_mul(
                        vw[:, h * HD:(h + 1) * HD],
                        v_sb[:, h * HD:(h + 1) * HD], e_new[:, h:h + 1])

            # ============ phase 2: attention ============
            with (tc.tile_pool(name="ps_attn", bufs=1, space="PSUM") as ps_attn,
                  tc.tile_pool(name="exp", bufs=4) as exp_pool):
                attn_ps = ps_attn.tile([128, 64], F32, tag="attn")
                nc.vector.memset(attn_ps[:], 0.0)

                attn_loop = (
                  tc.tile_pool(name="ps_kt", bufs=2, space="PSUM"),
                  tc.tile_pool(name="ps_sc", bufs=2, space="PSUM"),
                  tc.tile_pool(name="ps_dsm", bufs=2, space="PSUM"),
                  tc.tile_pool(name="kv", bufs=3),
                  tc.tile_pool(name="kv_v", bufs=3),
                  tc.tile_pool(name="ktsb", bufs=3))
                ps_kt, ps_sc, ps_dsm, kv_pool, kvv_pool, kt_pool = [
                    p.__enter__() for p in attn_loop]

                denom_flat = spool.tile([1, 32], F32, tag="denom_flat")
                nc.vector.memset(denom_flat[:], 0.0)
                CH = 4  # s-tiles per KV chunk (chunk = CH*128 seq positions)

                for b in range(B):
                    Tb = T[b]
                    if Tb == 0:
                        continue
                    n_ch = (Tb + CH - 1) // CH
                    Tev = Tb + (Tb & 1)
                    sc_all = ps_sc.tile([128, 256], F32, tag="sc",
                                        name=f"sc_b{b}")
                    exp_sb = exp_pool.tile([128, 256], F32R, tag="exp",
                                           name=f"exp_b{b}")
                    nc.vector.memset(exp_sb[:].bitcast(F32), 0.0)
                    for ci in range(n_ch):
                        nt = min(CH, Tb - ci * CH)
                        k_sb = kv_pool.tile([128, CH * HPC * HD], F32R,
                                            tag="k_sb", name=f"k_{b}_{ci}")
                        nc.sync.dma_start(
                            k_sb[:, 0:nt * HPC * HD]
                            .rearrange("p (i h d) -> p i (h d)", i=nt, h=HPC),
                            kc_d[b, ci * CH * 128:(ci * CH + nt) * 128, :, :]
                            .bitcast(F32R)
                            .rearrange("(i p) h d -> p i (h d)", p=128))
                        for h in range(HPC):
                            for j in range(nt):
                                t = ci * CH + j
                                kt_ps = ps_kt.tile([128, 128], F32, tag="kt")
                                nc.tensor.matmul(
                                    kt_ps[:].bitcast(F32R),
                                    k_sb[:, (j * HPC + h) * HD:
                                         (j * HPC + h + 1) * HD],
                                    ident_r[:, :], is_transpose=True,
                                    start=True, stop=True)
                                kt_sb = kt_pool.tile([128, 128], F32R,
                                                     tag="kt_sb")
                                if t % 2 == 0:
                                    nc.vector.tensor_copy(kt_sb[:], kt_ps[:])
                                else:
                                    nc.scalar.activation(kt_sb[:], kt_ps[:],
                                                         AF.Copy)
                                c = h * 64 + 2 * t
                                m = b * HPC + h
                                nc.tensor.matmul(
                                    sc_all[:, c:c + 2], kt_sb[:],
                                    qT[:, 2 * m:2 * m + 2],
                                    start=True, stop=True)
                    er = exp_sb[:].rearrange("p (h t l) -> p h t l",
                                             h=HPC, l=2)
                    sr = sc_all[:].rearrange("p (h t l) -> p h t l",
                                             h=HPC, l=2)
                    if Tb > 1:
                        nc.scalar.activation(er[:, :, 0:Tb - 1, 0],
                                             sr[:, :, 0:Tb - 1, 0], AF.Exp,
                                             bias=zbias[:, :])
                    nc.scalar.activation(er[:, :, Tb - 1:Tb, 0],
                                         sr[:, :, Tb - 1:Tb, 0], AF.Exp,
                                         bias=maskT[:, b:b + 1])
                    dsm = ps_dsm.tile([1, 128], F32, tag="dsm",
                                      name=f"dsm_b{b}")
                    nc.tensor.matmul(
                        dsm[0:1, 0:HPC * Tev], ones_col_r[:],
                        er[:, :, 0:Tev, 0], start=True, stop=True)
                    nc.vector.reduce_sum(
                        out=denom_flat[0:1, b * HPC:(b + 1) * HPC],
                        in_=dsm[0:1, 0:HPC * Tev]
                        .rearrange("o (h t) -> o h t", h=HPC),
                        axis=AX)
                    for ci in range(n_ch):
                        nt = min(CH, Tb - ci * CH)
                        v_sb = kvv_pool.tile([128, CH * HPC * HD], F32R,
                                             tag="v_sbt", name=f"v_{b}_{ci}")
                        nc.sync.dma_start(
                            v_sb[:, 0:nt * HPC * HD]
                            .rearrange("p (i h d) -> p i (h d)", i=nt, h=HPC),
                            vc_d[b, ci * CH * 128:(ci * CH + nt) * 128, :, :]
                            .bitcast(F32R)
                            .rearrange("(i p) h d -> p i (h d)", p=128))
                        for h in range(HPC):
                            for j in range(nt):
                                t = ci * CH + j
                                c = h * 64 + 2 * t
                                m = b * HPC + h
                                nc.tensor.matmul(
                                    attn_ps[:, 2 * m:2 * m + 2],
                                    v_sb[:, (j * HPC + h) * HD:
                                         (j * HPC + h + 1) * HD],
                                    exp_sb[:, c:c + 2],
                                    start=False, stop=(t == Tb - 1))

                for p in reversed(attn_loop):
                    p.__exit__(None, None, None)

                # new-token contribution + denominators + normalize
                with tc.tile_pool(name="ps_attn2", bufs=1,
                                  space="PSUM") as ps_attn2:
                    attn_new_ps = ps_attn2.tile([128, 32], F32, tag="attn_new")
                    for h in range(HPC):
                        nc.tensor.matmul(
                            attn_new_ps[:].rearrange(
                                "p (b h) -> p b h", h=HPC)[:, :, h],
                            vw[:, h * HD:(h + 1) * HD], ident[0:B, 0:B],
                            is_transpose=True, start=True, stop=True)
                    enew_dram = dram.tile([B, HPC], F32, tag="enew_d")
                    nc.sync.dma_start(enew_dram[:], e_new[:])
                    enew_flat = spool.tile([1, 32], F32, tag="enew_flat")
                    nc.sync.dma_start(
                        enew_flat[:],
                        enew_dram[:].rearrange("b h -> () (b h)"))
                    nc.vector.tensor_tensor(out=denom_flat[:],
                                            in0=denom_flat[:],
                                            in1=enew_flat[:], op=OP.add)
                    recip_flat = spool.tile([1, 32], F32, tag="recip_flat")
                    nc.vector.reciprocal(recip_flat[:], denom_flat[:])
                    bcast_ps = ps_attn2.tile([128, 32], F32, tag="bcast")
                    nc.tensor.matmul(bcast_ps[:], ones_row[:], recip_flat[:],
                                     start=True, stop=True)
                    attn_tot = spool.tile([128, 32], F32, tag="attn_tot")
                    nc.vector.tensor_copy(
                        attn_tot[:],
                        attn_ps[:].rearrange("p (m l) -> p m l", l=2)[:, :, 0])
                    nc.vector.tensor_tensor(out=attn_tot[:], in0=attn_tot[:],
                                            in1=attn_new_ps[:], op=OP.add)
                    attn_n = spool.tile([128, 32], MDT, tag="attn_n")
                    nc.vector.tensor_tensor(out=attn_n[:], in0=attn_tot[:],
                                            in1=bcast_ps[:], op=OP.mult)

            # ============ phase 3: dense + AR1 ============
            ar1_in = dram.tile([B, HID], F32, tag="ar1_in")
            ar1_out = dram.tile([B, HID], F32, tag="ar1_out")
            attn_nr = attn_n[:].rearrange("p (b h) -> p b h", h=HPC)
            with (tc.tile_pool(name="ps_o8a", bufs=8, space="PSUM") as o8a,
                  tc.tile_pool(name="wdp", bufs=2) as wdp):
                dps = [o8a.tile([B, 512], F32, tag="o8", name=f"dps{n}")
                       for n in range(8)]
                for h in range(HPC):
                    wd_sb = wdp.tile([128, HID], MDT, tag="wd")
                    nc.scalar.dma_start(wd_sb[:],
                                        g(wd_d[h * 128:(h + 1) * 128, :]))
                    for n in range(8):
                        nc.tensor.matmul(
                            dps[n][:], attn_nr[:, :, h],
                            wd_sb[:, n * 512:(n + 1) * 512],
                            start=(h == 0), stop=(h == HPC - 1))
                gout = spool.tile([B, HID], F32, tag="gemmout")
                for n in range(8):
                    nc.vector.tensor_copy(gout[:, n * 512:(n + 1) * 512],
                                          dps[n][:])
                nc.sync.dma_start(ar1_in[:], gout[:])
            nc.gpsimd.collective_compute(
                "AllReduce", mybir.AluOpType.add,
                replica_groups=[list(range(NC_))],
                ins=[ar1_in.opt()], outs=[ar1_out.opt()])

            x2 = spool.tile([B, HID], F32, tag="x2")
            ar_sb = spool.tile([B, HID], F32, tag="arx")
            nc.sync.dma_start(ar_sb[:], ar1_out[:])
            nc.vector.tensor_tensor(out=x2[:], in0=hidden[:], in1=ar_sb[:],
                                    op=OP.add)

            # ============ phase 4: LN2 + FFN + AR2 ============
            hg = spool.tile([B, F1PC // 2], F32, tag="arx", name="hg")
            with tc.tile_pool(name="ps_tp2", bufs=2, space="PSUM") as tp2:
                xc2 = ln_stats(x2)
                x2T = ln_transpose(xc2, ln2wT, ln2bT, tp2, "x2T")
            with (tc.tile_pool(name="ps_o8b", bufs=8, space="PSUM") as o8b,
                  tc.tile_pool(name="wf1p", bufs=3) as wf1p):
                hps = [o8b.tile([B, 512], F32, tag="o8",
                                name=f"hp{n}") for n in range(8)]
                for ki in range(32):
                    w_sb = wf1p.tile([128, HID], MDT, tag="wf1c")
                    eng = nc.sync if ki % 2 == 0 else nc.scalar
                    eng.dma_start(w_sb[:],
                                  g(wf1_d[ki * 128:(ki + 1) * 128, :]))
                    for n in range(8):
                        nc.tensor.matmul(
                            hps[n][:], x2T[:, ki * B:(ki + 1) * B],
                            w_sb[:, n * 512:(n + 1) * 512],
                            start=(ki == 0), stop=(ki == 31))
                for pair in range(4):
                    sil = spool.tile([B, 512], F32, tag="sil",
                                     name=f"sil{pair}")
                    nc.scalar.activation(sil[:], hps[2 * pair][:], AF.Silu,
                                         bias=zbias[0:B, :])
                    nc.vector.tensor_tensor(
                        out=hg[:, pair * 512:(pair + 1) * 512],
                        in0=sil[:], in1=hps[2 * pair + 1][:], op=OP.mult)

                hT = spool.tile([128, 16 * B], MDT, tag="hT")
                tpt = o8b.tile([128, 16 * B], F32, tag="o8", name="tpt")
                for i in range(16):
                    nc.tensor.matmul(
                        tpt[:, i * B:(i + 1) * B],
                        hg[:, i * 128:(i + 1) * 128], ident[0:B, 0:B],
                        is_transpose=True, start=True, stop=True)
                nc.vector.tensor_copy(hT[:], tpt[:])

            ar2_in = dram.tile([B, HID], F32, tag="ar2_in")
            ar2_out = dram.tile([B, HID], F32, tag="ar2_out")
            with (tc.tile_pool(name="ps_o8c", bufs=8, space="PSUM") as o8c,
                  tc.tile_pool(name="wf2p", bufs=2) as wf2p):
                fps = [o8c.tile([B, 512], F32, tag="o8", name=f"fps{n}")
                       for n in range(8)]
                for ki in range(16):
                    w_sb = wf2p.tile([128, HID], MDT, tag="wf2c")
                    nc.scalar.dma_start(w_sb[:],
                                        g(wf2_d[ki * 128:(ki + 1) * 128, :]))
                    for n in range(8):
                        nc.tensor.matmul(
                            fps[n][:], hT[:, ki * B:(ki + 1) * B],
                            w_sb[:, n * 512:(n + 1) * 512],
                            start=(ki == 0), stop=(ki == 15))
                gout2 = spool.tile([B, HID], F32, tag="gemmout")
                for n in range(8):
                    nc.vector.tensor_copy(gout2[:, n * 512:(n + 1) * 512],
                                          fps[n][:])
                nc.sync.dma_start(ar2_in[:], gout2[:])
            nc.gpsimd.collective_compute(
                "AllReduce", mybir.AluOpType.add,
                replica_groups=[list(range(NC_))],
                ins=[ar2_in.opt()], outs=[ar2_out.opt()])

            ar2_sb = spool.tile([B, HID], F32, tag="arx", name="ar2_sb")
            nc.sync.dma_start(ar2_sb[:], ar2_out[:])
            out_sb = spool.tile([B, HID], F32, tag="out_sb")
            nc.vector.tensor_tensor(out=out_sb[:], in0=x2[:], in1=ar2_sb[:],
                                    op=OP.add)
            nc.sync.dma_start(out_d[:], out_sb[:])

    nc.compile()
    return nc


def kernel(**inputs):
    from concourse.bass_utils import run_bass_kernel_spmd
    in_maps, T = _shard_inputs(inputs)
    nc = _build_program(T)
    trace = os.environ.get("KERNEL_TRACE", "0") == "1"
    tdir = os.environ.get("KERNEL_TRACE_DIR") or None
    res = run_bass_kernel_spmd(nc, in_maps, list(range(NC_)), trace=trace,
                               tmpdir=tdir)
    if trace and res.exec_time_ns is not None:
        print(f"HW exec time: {res.exec_time_ns} ns")
        print(f"mean exec time: {res.mean_exec_time_ns} ns "
              f"(max core {res.max_exec_time_core_id})")
        kernel.last_exec_time_ns = res.exec_time_ns
    return res.results[0]["out"]


if __name__ == "__main__":
    data = np.load("/tmp/ref_inputs.npz")
    inputs = {k: data[k] for k in data.files}
    ref = np.load("/tmp/ref_out.npy")
    out = kernel(**inputs)
    err = np.abs(out - ref).max()
    rel = err / np.abs(ref).max()
    print(f"absmax err: {err:.3e}  rel: {rel:.3e}")
